# revision 18
# baseline (speedup 1.0000x reference)
"""Trainium2 Bass kernel for nn_EncoderLayer_73315091743398.

The reference module's attention einsums ('hwink,hwijm->hwinm') sum their k/j
indices independently, so the whole attention block collapses to, per
(h,w)-chunk c and head i, over the flat q matrix qf = x@Wq.T + pe viewed as
(8192, 512) in raw (s,h,w) row order:

    u[s]  = sum_d qf[c*512+s, 64i+d]          (segment row sums)
    a     = softmax_s(u)
    v[d]  = sum_s a[s] * qf[c*512+s, 64i+d]
    row   = tile8(v) @ Wfc.T = v @ M,  M[d,:] = sum_b Wfc[:, 64b+d].T

and attn_out viewed (S,H,W,D) has row A[s'] = row_{c=s'//32, i=(s'%32)//4},
independent of (h,w).  Core k owns raw rows [k*1024,(k+1)*1024): these are
exactly attention chunks {2k, 2k+1} AND the residual/FFN rows for
s' in [64k, 64k+64), so the 8 cores run fully independent SPMD programs.

This version (vs the f32-heavy baseline):
  - pe is added inside the q-matmul PSUM group via a K=8 selector matmul
    (each 128-row tile uses 8 distinct pe rows, repeated 16x).
  - u is reduced directly from the q PSUM (exact f32; the peaked softmax
    needs it), while the SBUF copy of q used for v is bf16 via ScalarE.
  - residual adds z1/z2 are DVE tensor_adds producing bf16 tiles; LN stats
    and the normalize tensor_scalar then run in 2x bf16 mode.
  - o1 transpose for the FFN is a single SBUF->SBUF DMA-transpose per
    128-row tile (bf16 xbar path) instead of 4 PE transposes.
  - g1 is folded into W1, be1 into b1, (b2+be1) into the o1 tile, so the
    FFN stream is pure matmuls + one ScalarE relu per (half, ft).
  - per-column scale/bias (g2, be2) run on the otherwise-idle GpSimd.
"""

import math
import os
import sys
from contextlib import ExitStack

import numpy as np
import ml_dtypes  # noqa: F401  (registers bfloat16)

for _p in ("/opt/trn_rl_repo", "/root/.axon_site/_ro/trn_rl_repo"):
    if os.path.isdir(_p) and _p not in sys.path:
        sys.path.append(_p)

import concourse.bass as bass
import concourse.bacc as bacc
import concourse.mybir as mybir
import concourse.tile as tile
from concourse.bass_utils import run_bass_kernel_spmd

F32 = mybir.dt.float32
F32R = mybir.dt.float32r
BF16 = mybir.dt.bfloat16
AF = mybir.ActivationFunctionType
ALU = mybir.AluOpType
AX = mybir.AxisListType

S, H, W, D = 512, 4, 4, 512
NH, DEP, DFF = 8, 64, 2048
NCORES = 8
R = 1024          # rows per core of the flat (8192, 512) view
EPS = 1e-5

# CF (f32) column offsets: eye128, b1' packed [128, 16]
O_EYE, O_B1P = 0, 128
NCF = 144
# CB (bf16) column offsets
O_G1, O_G2, O_BE2, O_B1T, O_MST, O_E8, O_EYB = 0, 512, 1024, 1536, 2048, 2560, 3072
NCB = 3200
# CR8 (f32r, 8 partitions): selT [8,128], peT [8, 8*512]
NCR8 = 128 + 8 * D

_cached = {}


def build_nc():
    """Build the single-core SPMD Bass/Tile program (same program on all 8)."""
    nc = bacc.Bacc("TRN2", debug=False, target_bir_lowering=False)

    xT = nc.dram_tensor("xT", [D, R], F32R, kind="ExternalInput")
    xRb = nc.dram_tensor("xRb", [128, 8 * D], BF16, kind="ExternalInput")
    WqT = nc.dram_tensor("WqT", [D, D], F32R, kind="ExternalInput")
    W1T = nc.dram_tensor("W1T", [D, DFF], BF16, kind="ExternalInput")
    W2T = nc.dram_tensor("W2T", [DFF, D], BF16, kind="ExternalInput")
    CF = nc.dram_tensor("CF", [128, NCF], F32, kind="ExternalInput")
    CB = nc.dram_tensor("CB", [128, NCB], BF16, kind="ExternalInput")
    CR8 = nc.dram_tensor("CR8", [8, NCR8], F32R, kind="ExternalInput")
    out = nc.dram_tensor("out", [R, D], F32, kind="ExternalOutput")

    with ExitStack() as ctx:
        tc = ctx.enter_context(tile.TileContext(nc))
        cst = ctx.enter_context(tc.tile_pool(name="cst", bufs=1))
        xp = ctx.enter_context(tc.tile_pool(name="xp", bufs=1))
        qp = ctx.enter_context(tc.tile_pool(name="qp", bufs=1))
        hp = ctx.enter_context(tc.tile_pool(name="hp", bufs=1))
        wk = ctx.enter_context(tc.tile_pool(name="wk", bufs=2))
        ps = ctx.enter_context(tc.tile_pool(name="ps", bufs=1, space="PSUM"))

        # ---- loads: q-path inputs first (cr8/cf tiny, then wq/xq split into
        #      chunked DMAs so the first matmuls gate on minimal bytes);
        #      cb/xR/W1/W2 are gated later via a dummy write so their DMA
        #      doesn't steal HBM bandwidth from the critical q-path loads ----
        cr8 = cst.tile([8, NCR8], F32R, tag="cr8", name="cr8")
        nc.sync.dma_start(cr8[:], CR8[:])
        cf = cst.tile([128, NCF], F32, tag="cf", name="cf")
        nc.sync.dma_start(cf[:], CF[:])
        wq_all = cst.tile([128, 4 * D], F32R, tag="wq", name="wq_all")
        for t in range(4):
            nc.sync.dma_start(wq_all[:, t * D:(t + 1) * D],
                              WqT[t * 128:(t + 1) * 128, :])
        xq = [xp.tile([128, R], F32R, tag=f"dT{i}", name=f"xq{i}")
              for i in range(4)]
        for i in range(4):
            for hh in range(2):
                nc.sync.dma_start(xq[i][:, hh * 512:(hh + 1) * 512],
                                  xT[i * 128:(i + 1) * 128,
                                     hh * 512:(hh + 1) * 512])

        # deferred loads (tiles declared now, DMA issued after a gate write)
        cb = cst.tile([128, NCB], BF16, tag="cb", name="cb")
        xr_all = xp.tile([128, 8 * D], BF16, tag="xr", name="xr_all")
        w1_all = cst.tile([128, 4 * DFF], BF16, tag="w1", name="w1_all")
        w2_all = cst.tile([128, 16 * D], BF16, tag="w2", name="w2_all")

        eye_sb = cf[:, O_EYE:O_EYE + 128]
        G1cb = cb[:, O_G1:O_G1 + D]
        G2cb = cb[:, O_G2:O_G2 + D]
        BE2cb = cb[:, O_BE2:O_BE2 + D]
        B1Tcb = cb[:, O_B1T:O_B1T + D]
        Mstcb = cb[:, O_MST:O_MST + D]
        EYBcb = cb[:, O_EYB:O_EYB + 128]
        epsT = cst.tile([128, 1], F32, tag="eps", name="epsT")
        nc.vector.memset(epsT[:], EPS)

        # ---- PE warm-up + ACT table preload, all during the DMA wait ----
        # HAM un-throttles the PE clock (1.2->2.4 GHz) only after ~3.4us of
        # sustained matmul activity; burn that in on junk data while the
        # inputs stream in.  Likewise the Exp/Sqrt activation tables load
        # lazily (~1.3us) -- force them now, off the critical softmax path.
        warm_sb = cst.tile([128, D], BF16, tag="wrm", name="warm_sb")
        nc.vector.memset(warm_sb[:], 0.0)
        wps = ps.tile([128, D], F32, tag="vc", bufs=2)
        for _ in range(16):
            nc.tensor.matmul(wps[:], warm_sb[:, 0:128], warm_sb[:],
                             start=True, stop=True)
        junk = cst.tile([128, 1], F32, tag="junk", name="junk")
        nc.scalar.activation(junk[:], epsT[:], AF.Exp)
        nc.scalar.activation(junk[:], epsT[:], AF.Sqrt)

        q_sb = [qp.tile([128, D], BF16, tag=f"q{m}", name=f"qsb{m}")
                for m in range(8)]
        nrm1 = [qp.tile([128, D], BF16, tag=f"n1{m}", name=f"nrm1_{m}")
                for m in range(8)]
        o1_sb = [qp.tile([128, D], BF16, tag=f"o1{m}", name=f"o1sb{m}")
                 for m in range(8)]
        # o1T_all[p, m*512 + t*128 + y] = nrm1[m][y, t*128 + p]
        o1T_all = qp.tile([128, 8 * D], BF16, tag="oT", name="o1T_all")
        uT = [qp.tile([8, D], F32, tag=f"uT{c}", name=f"uTsb{c}")
              for c in range(2)]

        def q_stage(m):
            qps = ps.tile([128, D], F32, tag="mmA", bufs=2)
            i, mi = divmod(m, 2)
            for dt in range(4):
                nc.tensor.matmul(
                    qps[:],
                    xq[i][:, dt * 256 + mi * 128:dt * 256 + (mi + 1) * 128],
                    wq_all[:, dt * D:(dt + 1) * D],
                    start=(dt == 0), stop=False)
            # += sel.T @ pe8  (adds the 8 distinct pe rows of this tile)
            nc.tensor.matmul(
                qps[:], cr8[:, 0:128],
                cr8[:, 128 + m * D:128 + (m + 1) * D],
                start=False, stop=True)
            useg = wk.tile([128, 8], F32, tag="useg")
            nc.vector.tensor_reduce(
                useg[:], qps[:].rearrange("p (h d) -> p h d", h=8),
                axis=AX.X, op=ALU.add)
            nc.scalar.copy(q_sb[m][:], qps[:])          # f32 psum -> bf16 sbuf
            utp = ps.tile([8, 128], F32, tag="vc", bufs=2)
            nc.tensor.transpose(utp[:], useg[:], eye_sb)
            c, st = divmod(m, 4)
            nc.vector.tensor_copy(uT[c][:, st * 128:(st + 1) * 128], utp[:])

        def attn_softmax(c):
            mx = wk.tile([8, 1], F32, tag="mx")
            nc.vector.tensor_reduce(mx[:], uT[c][:], axis=AX.X, op=ALU.max)
            nmx = wk.tile([8, 1], F32, tag="nmx")
            nc.vector.tensor_scalar_mul(nmx[:], mx[:], -1.0)
            ex = wk.tile([8, D], F32, tag=f"ex{c}", bufs=1)
            ssum = wk.tile([8, 1], F32, tag="esum")
            nc.scalar.activation(ex[:], uT[c][:], AF.Exp, bias=nmx[:, :],
                                 accum_out=ssum[:])
            rcp = wk.tile([8, 1], F32, tag="ercp")
            nc.vector.reciprocal(rcp[:], ssum[:])
            nc.vector.tensor_scalar_mul(ex[:], ex[:], rcp[:])
            return ex

        def attn_prods(c, ex):
            aTss = []
            for st in range(4):
                atp = ps.tile([128, 8], F32, tag="vc", bufs=2)
                nc.tensor.transpose(atp[:], ex[:, st * 128:(st + 1) * 128],
                                    eye_sb[:8, :8])
                aTs = wk.tile([128, 8], BF16, tag=f"aT{c}{st}", bufs=1)
                nc.vector.tensor_copy(aTs[:], atp[:])
                aTss.append(aTs)
            return aTss

        def attn_la(c, aTss):
            vm = wk.tile([128, 8], BF16, tag=f"vm{c}", bufs=1)
            nc.vector.memset(vm[:], 0.0)
            for jt in range(4):
                # wsum[p, i] = sum_s q[s, 128*jt+p] * a_i[s]
                wsum = ps.tile([128, 8], F32, tag="vc", bufs=2)
                for st in range(4):
                    nc.tensor.matmul(
                        wsum[:], q_sb[c * 4 + st][:, jt * 128:(jt + 1) * 128],
                        aTss[st][:], start=(st == 0), stop=(st == 3))
                nc.vector.tensor_copy(vm[0:64, 2 * jt:2 * jt + 1],
                                      wsum[0:64, 2 * jt:2 * jt + 1])
                nc.vector.tensor_copy(vm[64:128, 2 * jt + 1:2 * jt + 2],
                                      wsum[64:128, 2 * jt + 1:2 * jt + 2])
            lap = ps.tile([8, D], F32, tag="vc", bufs=2)
            nc.tensor.matmul(lap[:], vm[:], Mstcb, start=True, stop=True)
            las = wk.tile([8, D], BF16, tag=f"las{c}", bufs=1)
            nc.vector.tensor_copy(las[:], lap[:])
            return las

        def attn_resid(c, las, jt):
            m = c * 4 + jt
            bcp = ps.tile([128, D], F32, tag="mmB", bufs=2)
            nc.tensor.matmul(bcp[:],
                             cb[0:8, O_E8 + jt * 128:O_E8 + (jt + 1) * 128],
                             las[:], start=True, stop=True)
            z1 = wk.tile([128, D], BF16, tag="z1")
            nc.vector.tensor_add(z1[:], bcp[:], xr_all[:, m * D:(m + 1) * D])
            st6 = wk.tile([128, 6], F32, tag="ls")
            nc.vector.bn_stats(st6[:], z1[:])
            mv = wk.tile([128, 2], F32, tag="lm")
            nc.vector.bn_aggr(mv[:], st6[:])
            sd = wk.tile([128, 1], F32, tag="lsd")
            nc.scalar.activation(sd[:], mv[:, 1:2], AF.Sqrt, bias=epsT[:, :])
            rsd = wk.tile([128, 1], F32, tag="lr")
            nc.vector.reciprocal(rsd[:], sd[:])
            nc.vector.tensor_scalar(nrm1[m][:], z1[:], mv[:, 0:1], rsd[:],
                                    op0=ALU.subtract, op1=ALU.mult)
            # o1 = nrm1*g1 + (b2+be1)   (GpSimd; consumed by the z2 add)
            nc.gpsimd.tensor_mul(o1_sb[m][:], nrm1[m][:], G1cb)
            nc.gpsimd.tensor_add(o1_sb[m][:], o1_sb[m][:], B1Tcb)
            # contiguous xbar dest: o1T_all[p, m*512 + t*128 + y] = nrm1[y, t*128+p]
            # alternate HWDGE queues (sync/scalar) so the ~1.2us descriptor
            # generation of consecutive transposes overlaps
            eng = nc.sync if m % 2 == 0 else nc.scalar
            eng.dma_start_transpose(
                o1T_all[:, m * D:(m + 1) * D].rearrange("p (t y) -> p t y", t=4),
                nrm1[m][:])

        h1map = {}

        # strided rhs view: oTr[p, m, t, y] = o1T_all[p, m*512 + t*128 + y]
        oTr = o1T_all[:].rearrange("p (m t y) -> p m t y", m=8, t=4)

        def ffn_h1(h, ft):
            p1 = ps.tile([128, D], F32, tag="mmA", bufs=2)
            for dt in range(4):
                nc.tensor.matmul(
                    p1[:],
                    w1_all[:, dt * DFF + ft * 128:dt * DFF + (ft + 1) * 128],
                    oTr[:, h * 4:(h + 1) * 4, dt, :],
                    start=(dt == 0), stop=(dt == 3))
            h1t = hp.tile([128, D], BF16, tag=f"h1_{ft}", bufs=2,
                          name=f"h1_{h}_{ft}")
            nc.scalar.activation(h1t[:], p1[:], AF.Relu,
                                 bias=cf[:, O_B1P + ft:O_B1P + ft + 1])
            h1map[(h, ft)] = h1t

        def ffn_rm(m):
            h, rm = divmod(m, 4)
            tail = m >= 6
            p2 = ps.tile([128, D], F32, tag="mmC", bufs=2)
            for ft in range(16):
                nc.tensor.matmul(
                    p2[:], h1map[(h, ft)][:, rm * 128:(rm + 1) * 128],
                    w2_all[:, ft * D:(ft + 1) * D],
                    start=(ft == 0), stop=(ft == 15 and not tail))
            if tail:
                # z2 += o1 on the (tail-idle) PE; LN2 reads PSUM directly,
                # shortening the post-matmul serial chain of the last tiles
                nc.tensor.matmul(p2[:], EYBcb, o1_sb[m][:],
                                 start=False, stop=True)
                z2 = p2
            else:
                z2t = wk.tile([128, D], BF16, tag="z2")
                nc.vector.tensor_add(z2t[:], p2[:], o1_sb[m][:])
                z2 = z2t
            st6 = wk.tile([128, 6], F32, tag="ls")
            nc.vector.bn_stats(st6[:], z2[:])
            mv = wk.tile([128, 2], F32, tag="lm")
            nc.vector.bn_aggr(mv[:], st6[:])
            sd = wk.tile([128, 1], F32, tag="lsd")
            nc.scalar.activation(sd[:], mv[:, 1:2], AF.Sqrt, bias=epsT[:, :])
            rsd = wk.tile([128, 1], F32, tag="lr")
            nc.vector.reciprocal(rsd[:], sd[:])
            nrm2 = wk.tile([128, D], BF16, tag="n2")
            nc.vector.tensor_scalar(nrm2[:], z2[:], mv[:, 0:1], rsd[:],
                                    op0=ALU.subtract, op1=ALU.mult)
            tg = wk.tile([128, D], BF16, tag="tg")
            yt = wk.tile([128, D], F32, tag="yt")
            if tail:
                nc.vector.tensor_mul(tg[:], nrm2[:], G2cb)
                nc.vector.tensor_add(yt[:], tg[:], BE2cb)
            else:
                nc.gpsimd.tensor_mul(tg[:], nrm2[:], G2cb)
                nc.gpsimd.tensor_add(yt[:], tg[:], BE2cb)
            nc.sync.dma_start(out[m * 128:(m + 1) * 128, :], yt[:])

        def gate_dma(tile_t, tile_ap, dram):
            """Order a big load after earlier compute: a 1-elem dummy write
            creates a WAR dependency the DMA must wait on."""
            nc.vector.memset(tile_t[0:1, 0:1], 0.0)
            nc.sync.dma_start(tile_ap, dram)

        # ---------------- schedule ----------------
        # ungated bulk loads last in issue order (cb, xr); W1/W2 gated
        nc.sync.dma_start(cb[:], CB[:])
        nc.sync.dma_start(xr_all[:], xRb[:])
        q_stage(0)
        q_stage(1)
        q_stage(2)
        q_stage(3)
        gate_dma(w1_all, w1_all[:].rearrange("p (t j) -> p t j", t=4),
                 W1T.rearrange("(t p) j -> p t j", p=128))
        a0 = attn_softmax(0)
        aT0 = attn_prods(0, a0)
        gate_dma(w2_all, w2_all[:].rearrange("p (t j) -> p t j", t=16),
                 W2T.rearrange("(t p) j -> p t j", p=128))
        q_stage(4)
        las0 = attn_la(0, aT0)
        q_stage(5)
        attn_resid(0, las0, 0)
        attn_resid(0, las0, 1)
        q_stage(6)
        attn_resid(0, las0, 2)
        attn_resid(0, las0, 3)
        q_stage(7)
        a1 = attn_softmax(1)
        aT1 = attn_prods(1, a1)
        las1 = attn_la(1, aT1)
        attn_resid(1, las1, 0)
        attn_resid(1, las1, 1)
        for ft in range(8):
            ffn_h1(0, ft)
        attn_resid(1, las1, 2)
        for ft in range(8, 12):
            ffn_h1(0, ft)
        attn_resid(1, las1, 3)
        for ft in range(12, 16):
            ffn_h1(0, ft)
        ffn_rm(0)
        for ft in range(0, 4):
            ffn_h1(1, ft)
        ffn_rm(1)
        for ft in range(4, 8):
            ffn_h1(1, ft)
        ffn_rm(2)
        for ft in range(8, 12):
            ffn_h1(1, ft)
        ffn_rm(3)
        for ft in range(12, 16):
            ffn_h1(1, ft)
        for m in range(4, 8):
            ffn_rm(m)

    nc.compile()
    return nc


def _round_f32r(a):
    b = np.ascontiguousarray(a, dtype=np.float32).view(np.uint32)
    out = (b + 0x7FF + ((b >> 12) & 1)) & np.uint32(0xFFFFF000)
    return out.view(np.float32)


def _pe_table():
    pos = np.arange(S, dtype=np.float32)[:, None]
    div = np.exp(np.arange(0, D, 2, dtype=np.float32) * (-math.log(10000.0) / D))
    ang = pos * div
    pe = np.zeros((S, D), np.float32)
    pe[:, 0::2] = np.sin(ang)
    pe[:, 1::2] = np.cos(ang)
    return pe


def make_in_maps(x, Wq, Wfc, W1, b1, W2, b2, g1, be1, g2, be2):
    f32 = lambda a: np.ascontiguousarray(a, dtype=np.float32)
    bfc = lambda a: np.ascontiguousarray(np.asarray(f32(a), dtype="bfloat16"))
    xf = f32(x).reshape(S * H * W, D)
    pe = _pe_table()
    M2 = f32(Wfc).reshape(D, NH, DEP).sum(axis=1).T          # (64, 512)
    Mstk = np.concatenate([M2, M2], axis=0)                  # (128, 512)

    CF = np.zeros((128, NCF), np.float32)
    CF[:, O_EYE:O_EYE + 128] = np.eye(128, dtype=np.float32)
    b1p = f32(b1) + f32(W1) @ f32(be1)                       # be1 folded
    CF[:, O_B1P:O_B1P + 16] = b1p.reshape(16, 128).T

    CB = np.zeros((128, NCB), np.float32)
    CB[:, O_G1:O_G1 + D] = np.tile(f32(g1), (128, 1))
    CB[:, O_G2:O_G2 + D] = np.tile(f32(g2), (128, 1))
    CB[:, O_BE2:O_BE2 + D] = np.tile(f32(be2), (128, 1))
    CB[:, O_B1T:O_B1T + D] = np.tile(f32(b2) + f32(be1), (128, 1))
    CB[:, O_MST:O_MST + D] = Mstk
    CB[:, O_EYB:O_EYB + 128] = np.eye(128, dtype=np.float32)
    for jt in range(4):
        for p in range(128):
            CB[2 * jt + p // 64, O_E8 + jt * 128 + p] = 1.0

    # selT [8, 128]: selT[j, rr] = 1 iff rr//16 == j; peT per m-tile
    shared = dict(
        WqT=_round_f32r(Wq.T),
        W1T=bfc(f32(W1) * f32(g1)[None, :]).T.copy(),        # g1 folded
        W2T=bfc(f32(W2).T),
        CF=CF, CB=bfc(CB),
    )
    selT = np.zeros((8, 128), np.float32)
    for rr in range(128):
        selT[rr // 16, rr] = 1.0

    maps = []
    for k in range(NCORES):
        sl = xf[k * R:(k + 1) * R]
        m = dict(shared)
        slT = _round_f32r(sl.T)
        # xq layout: row-block i = m-pair (2i, 2i+1); columns (dt, mi, c)
        arr = slT.reshape(4, 128, 4, 2, 128)        # (t, p, i, mi, c)
        arr = arr.transpose(2, 1, 0, 3, 4)          # (i, p, t, mi, c)
        m["xT"] = np.ascontiguousarray(arr.reshape(512, 1024))
        # xRb[p, t*512+d] = x[t*128+p, d]
        m["xRb"] = bfc(sl.reshape(8, 128, D).transpose(1, 0, 2)
                       .reshape(128, 8 * D))
        cr8 = np.zeros((8, NCR8), np.float32)
        cr8[:, 0:128] = selT
        pe_loc = pe[k * 64:(k + 1) * 64]            # (64, 512)
        cr8[:, 128:] = _round_f32r(pe_loc.reshape(8, 8, D)
                                   .transpose(1, 0, 2).reshape(8, 8 * D))
        m["CR8"] = _round_f32r(cr8)
        maps.append(m)
    return maps


def kernel(x, Wq, Wfc, W1, b1, W2, b2, g1, be1, g2, be2, _results_hook=None,
           _trace=False, _tmpdir=None):
    if "nc" not in _cached:
        _cached["nc"] = build_nc()
    nc = _cached["nc"]
    in_maps = make_in_maps(x, Wq, Wfc, W1, b1, W2, b2, g1, be1, g2, be2)
    res = run_bass_kernel_spmd(nc, in_maps, list(range(NCORES)),
                               trace=_trace, tmpdir=_tmpdir)
    if _results_hook is not None:
        _results_hook(res)
    y = np.concatenate([res.results[k]["out"] for k in range(NCORES)], axis=0)
    return y.reshape(S, H, W, D)


# revision 22
# speedup vs baseline: 1.1013x; 1.1013x over previous
"""Trainium2 Bass kernel for nn_EncoderLayer_73315091743398.

The reference module's attention einsums ('hwink,hwijm->hwinm') sum their k/j
indices independently, so the whole attention block collapses to, per
(h,w)-chunk c and head i, over the flat q matrix qf = x@Wq.T + pe viewed as
(8192, 512) in raw (s,h,w) row order:

    u[s]  = sum_d qf[c*512+s, 64i+d]          (segment row sums)
    a     = softmax_s(u)
    v[d]  = sum_s a[s] * qf[c*512+s, 64i+d]
    row   = tile8(v) @ Wfc.T = v @ M,  M[d,:] = sum_b Wfc[:, 64b+d].T

and attn_out viewed (S,H,W,D) has row A[s'] = row_{c=s'//32, i=(s'%32)//4},
independent of (h,w).  Core k owns raw rows [k*1024,(k+1)*1024): these are
exactly attention chunks {2k, 2k+1} AND the residual/FFN rows for
s' in [64k, 64k+64), so the 8 cores run fully independent SPMD programs.

This version (vs the f32-heavy baseline):
  - pe is added inside the q-matmul PSUM group via a K=8 selector matmul
    (each 128-row tile uses 8 distinct pe rows, repeated 16x).
  - u is reduced directly from the q PSUM (exact f32; the peaked softmax
    needs it), while the SBUF copy of q used for v is bf16 via ScalarE.
  - residual adds z1/z2 are DVE tensor_adds producing bf16 tiles; LN stats
    and the normalize tensor_scalar then run in 2x bf16 mode.
  - o1 transpose for the FFN is a single SBUF->SBUF DMA-transpose per
    128-row tile (bf16 xbar path) instead of 4 PE transposes.
  - g1 is folded into W1, be1 into b1, (b2+be1) into the o1 tile, so the
    FFN stream is pure matmuls + one ScalarE relu per (half, ft).
  - per-column scale/bias (g2, be2) run on the otherwise-idle GpSimd.
"""

import math
import os
import sys
from contextlib import ExitStack

import numpy as np
import ml_dtypes  # noqa: F401  (registers bfloat16)

for _p in ("/opt/trn_rl_repo", "/root/.axon_site/_ro/trn_rl_repo"):
    if os.path.isdir(_p) and _p not in sys.path:
        sys.path.append(_p)

import concourse.bass as bass
import concourse.bacc as bacc
import concourse.mybir as mybir
import concourse.tile as tile
from concourse.bass_utils import run_bass_kernel_spmd

F32 = mybir.dt.float32
F32R = mybir.dt.float32r
BF16 = mybir.dt.bfloat16
AF = mybir.ActivationFunctionType
ALU = mybir.AluOpType
AX = mybir.AxisListType

S, H, W, D = 512, 4, 4, 512
NH, DEP, DFF = 8, 64, 2048
NCORES = 8
R = 1024          # rows per core of the flat (8192, 512) view
EPS = 1e-5

# CF (f32) column offsets: eye128, b1' packed [128, 16]
O_EYE, O_B1P = 0, 128
NCF = 144
# CB (bf16) column offsets
O_G1, O_G2, O_BE2, O_B1T, O_MST, O_E8, O_EYB = 0, 512, 1024, 1536, 2048, 2560, 3072
NCB = 3200
# CR8 (f32r, 8 partitions): selT [8,128], peT [8, 8*512]
NCR8 = 128 + 8 * D

_cached = {}


def build_nc():
    """Build the single-core SPMD Bass/Tile program (same program on all 8)."""
    nc = bacc.Bacc("TRN2", debug=False, target_bir_lowering=False)

    xT = nc.dram_tensor("xT", [D, R], F32R, kind="ExternalInput")
    xRb = nc.dram_tensor("xRb", [128, 8 * D], BF16, kind="ExternalInput")
    WqT = nc.dram_tensor("WqT", [D, D], F32R, kind="ExternalInput")
    W1T = nc.dram_tensor("W1T", [D, DFF], BF16, kind="ExternalInput")
    W2T = nc.dram_tensor("W2T", [DFF, D], BF16, kind="ExternalInput")
    CF = nc.dram_tensor("CF", [128, NCF], F32, kind="ExternalInput")
    CB = nc.dram_tensor("CB", [128, NCB], BF16, kind="ExternalInput")
    CR8 = nc.dram_tensor("CR8", [8, NCR8], F32R, kind="ExternalInput")
    out = nc.dram_tensor("out", [R, D], F32, kind="ExternalOutput")

    with ExitStack() as ctx:
        tc = ctx.enter_context(tile.TileContext(nc))
        cst = ctx.enter_context(tc.tile_pool(name="cst", bufs=1))
        xp = ctx.enter_context(tc.tile_pool(name="xp", bufs=1))
        qp = ctx.enter_context(tc.tile_pool(name="qp", bufs=1))
        hp = ctx.enter_context(tc.tile_pool(name="hp", bufs=1))
        wk = ctx.enter_context(tc.tile_pool(name="wk", bufs=2))
        ps = ctx.enter_context(tc.tile_pool(name="ps", bufs=1, space="PSUM"))

        # ---- loads: q-path inputs first (cr8/cf tiny, then wq/xq split into
        #      chunked DMAs so the first matmuls gate on minimal bytes);
        #      cb/xR/W1/W2 are gated later via a dummy write so their DMA
        #      doesn't steal HBM bandwidth from the critical q-path loads ----
        cr8 = cst.tile([8, NCR8], F32R, tag="cr8", name="cr8")
        nc.sync.dma_start(cr8[:], CR8[:])
        cf = cst.tile([128, NCF], F32, tag="cf", name="cf")
        nc.sync.dma_start(cf[:], CF[:])
        # wq chunk t + xq0 quarter t arrive together -> the zigzagged first
        # two q-tiles can start their dt=t matmuls as soon as each pair lands
        wq_all = cst.tile([128, 4 * D], F32R, tag="wq", name="wq_all")
        xq = [xp.tile([128, R], F32R, tag=f"dT{i}", name=f"xq{i}")
              for i in range(4)]
        for t in range(4):
            nc.sync.dma_start(wq_all[:, t * D:(t + 1) * D],
                              WqT[t * 128:(t + 1) * 128, :])
            nc.sync.dma_start(xq[0][:, t * 256:(t + 1) * 256],
                              xT[0:128, t * 256:(t + 1) * 256])
        for i in range(1, 4):
            nc.sync.dma_start(xq[i][:], xT[i * 128:(i + 1) * 128, :])

        # deferred loads (tiles declared now, DMA issued after a gate write)
        cb = cst.tile([128, NCB], BF16, tag="cb", name="cb")
        xr_all = xp.tile([128, 8 * D], BF16, tag="xr", name="xr_all")
        w1_all = cst.tile([128, 4 * DFF], BF16, tag="w1", name="w1_all")
        w2_all = cst.tile([128, 16 * D], BF16, tag="w2", name="w2_all")

        eye_sb = cf[:, O_EYE:O_EYE + 128]
        G1cb = cb[:, O_G1:O_G1 + D]
        G2cb = cb[:, O_G2:O_G2 + D]
        BE2cb = cb[:, O_BE2:O_BE2 + D]
        B1Tcb = cb[:, O_B1T:O_B1T + D]
        Mstcb = cb[:, O_MST:O_MST + D]
        EYBcb = cb[:, O_EYB:O_EYB + 128]
        epsT = cst.tile([128, 1], F32, tag="eps", name="epsT")
        nc.vector.memset(epsT[:], EPS)

        # ---- PE warm-up + ACT table preload, all during the DMA wait ----
        # HAM un-throttles the PE clock (1.2->2.4 GHz) only after ~3.4us of
        # sustained matmul activity; burn that in on junk data while the
        # inputs stream in.  Likewise the Exp/Sqrt activation tables load
        # lazily (~1.3us) -- force them now, off the critical softmax path.
        warm_sb = cst.tile([128, D], BF16, tag="wrm", name="warm_sb")
        nc.vector.memset(warm_sb[:], 0.0)
        wps = ps.tile([128, D], F32, tag="vc", bufs=2)
        for _ in range(16):
            nc.tensor.matmul(wps[:], warm_sb[:, 0:128], warm_sb[:],
                             start=True, stop=True)
        junk = cst.tile([128, 1], F32, tag="junk", name="junk")
        nc.scalar.activation(junk[:], epsT[:], AF.Exp)
        nc.scalar.activation(junk[:], epsT[:], AF.Sqrt)

        q_sb = [qp.tile([128, D], BF16, tag=f"q{m}", name=f"qsb{m}")
                for m in range(8)]
        nrm1 = [qp.tile([128, D], BF16, tag=f"n1{m}", name=f"nrm1_{m}")
                for m in range(8)]
        o1_sb = [qp.tile([128, D], BF16, tag=f"o1{m}", name=f"o1sb{m}")
                 for m in range(8)]
        # o1T_all[p, m*512 + t*128 + y] = nrm1[m][y, t*128 + p]
        o1T_all = qp.tile([128, 8 * D], BF16, tag="oT", name="o1T_all")
        uT = [qp.tile([8, D], F32, tag=f"uT{c}", name=f"uTsb{c}")
              for c in range(2)]

        qps_map = {}

        def q_mm(m):
            qps = ps.tile([128, D], F32, tag="mmQ", bufs=2)
            qps_map[m] = qps
            i, mi = divmod(m, 2)
            for dt in range(4):
                nc.tensor.matmul(
                    qps[:],
                    xq[i][:, dt * 256 + mi * 128:dt * 256 + (mi + 1) * 128],
                    wq_all[:, dt * D:(dt + 1) * D],
                    start=(dt == 0), stop=False)
            # += sel.T @ pe8  (adds the 8 distinct pe rows of this tile)
            nc.tensor.matmul(
                qps[:], cr8[:, 0:128],
                cr8[:, 128 + m * D:128 + (m + 1) * D],
                start=False, stop=True)

        def q_mm_pair(ma, mb):
            """Zigzag two q tiles' accumulation groups across two PSUM banks
            at dt granularity, so matmuls start as each wq/xq chunk lands."""
            qa = ps.tile([128, D], F32, tag="mmQ", bufs=2)
            qb = ps.tile([128, D], F32, tag="mmQ", bufs=2)
            qps_map[ma], qps_map[mb] = qa, qb
            for dt in range(4):
                for m, qp_ in ((ma, qa), (mb, qb)):
                    i, mi = divmod(m, 2)
                    nc.tensor.matmul(
                        qp_[:],
                        xq[i][:, dt * 256 + mi * 128:dt * 256 + (mi + 1) * 128],
                        wq_all[:, dt * D:(dt + 1) * D],
                        start=(dt == 0), stop=False)
            for m, qp_ in ((ma, qa), (mb, qb)):
                nc.tensor.matmul(
                    qp_[:], cr8[:, 0:128],
                    cr8[:, 128 + m * D:128 + (m + 1) * D],
                    start=False, stop=True)

        def q_fin(m):
            qps = qps_map[m]
            useg = wk.tile([128, 8], F32, tag="useg")
            nc.vector.tensor_reduce(
                useg[:], qps[:].rearrange("p (h d) -> p h d", h=8),
                axis=AX.X, op=ALU.add)
            nc.scalar.copy(q_sb[m][:], qps[:])          # f32 psum -> bf16 sbuf
            utp = ps.tile([8, 128], F32, tag="vc", bufs=2)
            nc.tensor.transpose(utp[:], useg[:], eye_sb)
            c, st = divmod(m, 4)
            nc.vector.tensor_copy(uT[c][:, st * 128:(st + 1) * 128], utp[:])

        def attn_softmax(c):
            mx = wk.tile([8, 1], F32, tag="mx")
            nc.vector.tensor_reduce(mx[:], uT[c][:], axis=AX.X, op=ALU.max)
            nmx = wk.tile([8, 1], F32, tag="nmx")
            nc.vector.tensor_scalar_mul(nmx[:], mx[:], -1.0)
            ex = wk.tile([8, D], F32, tag=f"ex{c}", bufs=1)
            ssum = wk.tile([8, 1], F32, tag="esum")
            nc.scalar.activation(ex[:], uT[c][:], AF.Exp, bias=nmx[:, :],
                                 accum_out=ssum[:])
            rcp = wk.tile([8, 1], F32, tag="ercp")
            nc.vector.reciprocal(rcp[:], ssum[:])
            nc.vector.tensor_scalar_mul(ex[:], ex[:], rcp[:])
            return ex

        def attn_prods(c, ex):
            aTss = []
            for st in range(4):
                atp = ps.tile([128, 8], F32, tag="vc", bufs=2)
                nc.tensor.transpose(atp[:], ex[:, st * 128:(st + 1) * 128],
                                    eye_sb[:8, :8])
                aTs = wk.tile([128, 8], BF16, tag=f"aT{c}{st}", bufs=1)
                nc.vector.tensor_copy(aTs[:], atp[:])
                aTss.append(aTs)
            return aTss

        def attn_la(c, aTss):
            vm = wk.tile([128, 8], BF16, tag=f"vm{c}", bufs=1)
            nc.vector.memset(vm[:], 0.0)
            for jt in range(4):
                # wsum[p, i] = sum_s q[s, 128*jt+p] * a_i[s]
                wsum = ps.tile([128, 8], F32, tag="vc", bufs=2)
                for st in range(4):
                    nc.tensor.matmul(
                        wsum[:], q_sb[c * 4 + st][:, jt * 128:(jt + 1) * 128],
                        aTss[st][:], start=(st == 0), stop=(st == 3))
                nc.vector.tensor_copy(vm[0:64, 2 * jt:2 * jt + 1],
                                      wsum[0:64, 2 * jt:2 * jt + 1])
                nc.vector.tensor_copy(vm[64:128, 2 * jt + 1:2 * jt + 2],
                                      wsum[64:128, 2 * jt + 1:2 * jt + 2])
            lap = ps.tile([8, D], F32, tag="vc", bufs=2)
            nc.tensor.matmul(lap[:], vm[:], Mstcb, start=True, stop=True)
            las = wk.tile([8, D], BF16, tag=f"las{c}", bufs=1)
            nc.vector.tensor_copy(las[:], lap[:])
            return las

        def attn_resid(c, las, jt):
            m = c * 4 + jt
            bcp = ps.tile([128, D], F32, tag="mmB", bufs=2)
            nc.tensor.matmul(bcp[:],
                             cb[0:8, O_E8 + jt * 128:O_E8 + (jt + 1) * 128],
                             las[:], start=True, stop=True)
            z1 = wk.tile([128, D], BF16, tag="z1")
            nc.vector.tensor_add(z1[:], bcp[:], xr_all[:, m * D:(m + 1) * D])
            st6 = wk.tile([128, 6], F32, tag="ls")
            nc.vector.bn_stats(st6[:], z1[:])
            mv = wk.tile([128, 2], F32, tag="lm")
            nc.vector.bn_aggr(mv[:], st6[:])
            sd = wk.tile([128, 1], F32, tag="lsd")
            nc.scalar.activation(sd[:], mv[:, 1:2], AF.Sqrt, bias=epsT[:, :])
            rsd = wk.tile([128, 1], F32, tag="lr")
            nc.vector.reciprocal(rsd[:], sd[:])
            nc.vector.tensor_scalar(nrm1[m][:], z1[:], mv[:, 0:1], rsd[:],
                                    op0=ALU.subtract, op1=ALU.mult)
            # o1 = nrm1*g1 + (b2+be1)   (GpSimd; consumed by the z2 add)
            nc.gpsimd.tensor_mul(o1_sb[m][:], nrm1[m][:], G1cb)
            nc.gpsimd.tensor_add(o1_sb[m][:], o1_sb[m][:], B1Tcb)
            # contiguous xbar dest: o1T_all[p, m*512 + t*128 + y] = nrm1[y, t*128+p]
            # alternate HWDGE queues (sync/scalar) so the ~1.2us descriptor
            # generation of consecutive transposes overlaps
            eng = nc.sync if m % 2 == 0 else nc.scalar
            eng.dma_start_transpose(
                o1T_all[:, m * D:(m + 1) * D].rearrange("p (t y) -> p t y", t=4),
                nrm1[m][:])

        h1map = {}

        # strided rhs view: oTr[p, m, t, y] = o1T_all[p, m*512 + t*128 + y]
        oTr = o1T_all[:].rearrange("p (m t y) -> p m t y", m=8, t=4)

        def ffn_h1(h, ft):
            p1 = ps.tile([128, D], F32, tag="mmA", bufs=2)
            for dt in range(4):
                nc.tensor.matmul(
                    p1[:],
                    w1_all[:, dt * DFF + ft * 128:dt * DFF + (ft + 1) * 128],
                    oTr[:, h * 4:(h + 1) * 4, dt, :],
                    start=(dt == 0), stop=(dt == 3))
            h1t = hp.tile([128, D], BF16, tag=f"h1_{ft}", bufs=2,
                          name=f"h1_{h}_{ft}")
            nc.scalar.activation(h1t[:], p1[:], AF.Relu,
                                 bias=cf[:, O_B1P + ft:O_B1P + ft + 1])
            h1map[(h, ft)] = h1t

        def ffn_rm(m):
            h, rm = divmod(m, 4)
            tail = m >= 6
            p2 = ps.tile([128, D], F32, tag="mmB", bufs=2)
            for ft in range(16):
                nc.tensor.matmul(
                    p2[:], h1map[(h, ft)][:, rm * 128:(rm + 1) * 128],
                    w2_all[:, ft * D:(ft + 1) * D],
                    start=(ft == 0), stop=(ft == 15 and not tail))
            if tail:
                # z2 += o1 on the (tail-idle) PE; LN2 reads PSUM directly,
                # shortening the post-matmul serial chain of the last tiles
                nc.tensor.matmul(p2[:], EYBcb, o1_sb[m][:],
                                 start=False, stop=True)
                z2 = p2
            else:
                z2t = wk.tile([128, D], BF16, tag="z2")
                nc.vector.tensor_add(z2t[:], p2[:], o1_sb[m][:])
                z2 = z2t
            st6 = wk.tile([128, 6], F32, tag="ls")
            nc.vector.bn_stats(st6[:], z2[:])
            mv = wk.tile([128, 2], F32, tag="lm")
            nc.vector.bn_aggr(mv[:], st6[:])
            sd = wk.tile([128, 1], F32, tag="lsd")
            nc.scalar.activation(sd[:], mv[:, 1:2], AF.Sqrt, bias=epsT[:, :])
            rsd = wk.tile([128, 1], F32, tag="lr")
            nc.vector.reciprocal(rsd[:], sd[:])
            nrm2 = wk.tile([128, D], BF16, tag="n2")
            nc.vector.tensor_scalar(nrm2[:], z2[:], mv[:, 0:1], rsd[:],
                                    op0=ALU.subtract, op1=ALU.mult)
            tg = wk.tile([128, D], BF16, tag="tg")
            yt = wk.tile([128, D], F32, tag="yt")
            if tail:
                nc.vector.tensor_mul(tg[:], nrm2[:], G2cb)
                nc.vector.tensor_add(yt[:], tg[:], BE2cb)
            else:
                nc.gpsimd.tensor_mul(tg[:], nrm2[:], G2cb)
                nc.gpsimd.tensor_add(yt[:], tg[:], BE2cb)
            nc.sync.dma_start(out[m * 128:(m + 1) * 128, :], yt[:])

        def gate_dma(tile_t, tile_ap, dram):
            """Order a big load after earlier compute: a 1-elem dummy write
            creates a WAR dependency the DMA must wait on."""
            nc.vector.memset(tile_t[0:1, 0:1], 0.0)
            nc.sync.dma_start(tile_ap, dram)

        # ---------------- schedule ----------------
        # ungated bulk loads last in issue order (cb, xr); W1/W2 gated
        nc.sync.dma_start(cb[:], CB[:])
        nc.sync.dma_start(xr_all[:], xRb[:])
        q_mm_pair(0, 1)
        q_fin(0)
        q_fin(1)
        q_mm(2)
        q_fin(2)
        q_mm(3)
        q_fin(3)
        gate_dma(w1_all, w1_all[:].rearrange("p (t j) -> p t j", t=4),
                 W1T.rearrange("(t p) j -> p t j", p=128))
        a0 = attn_softmax(0)
        aT0 = attn_prods(0, a0)
        gate_dma(w2_all, w2_all[:].rearrange("p (t j) -> p t j", t=16),
                 W2T.rearrange("(t p) j -> p t j", p=128))
        q_mm(4)
        q_fin(4)
        las0 = attn_la(0, aT0)
        q_mm(5)
        q_fin(5)
        attn_resid(0, las0, 0)
        attn_resid(0, las0, 1)
        q_mm(6)
        q_mm(7)
        q_fin(6)
        q_fin(7)
        attn_resid(0, las0, 2)
        attn_resid(0, las0, 3)
        a1 = attn_softmax(1)
        aT1 = attn_prods(1, a1)
        las1 = attn_la(1, aT1)
        attn_resid(1, las1, 0)
        attn_resid(1, las1, 1)
        for ft in range(8):
            ffn_h1(0, ft)
        attn_resid(1, las1, 2)
        for ft in range(8, 12):
            ffn_h1(0, ft)
        attn_resid(1, las1, 3)
        for ft in range(12, 16):
            ffn_h1(0, ft)
        ffn_rm(0)
        for ft in range(0, 4):
            ffn_h1(1, ft)
        ffn_rm(1)
        for ft in range(4, 8):
            ffn_h1(1, ft)
        ffn_rm(2)
        for ft in range(8, 12):
            ffn_h1(1, ft)
        ffn_rm(3)
        for ft in range(12, 16):
            ffn_h1(1, ft)
        for m in range(4, 8):
            ffn_rm(m)

    nc.compile()
    return nc


def _round_f32r(a):
    b = np.ascontiguousarray(a, dtype=np.float32).view(np.uint32)
    out = (b + 0x7FF + ((b >> 12) & 1)) & np.uint32(0xFFFFF000)
    return out.view(np.float32)


def _pe_table():
    pos = np.arange(S, dtype=np.float32)[:, None]
    div = np.exp(np.arange(0, D, 2, dtype=np.float32) * (-math.log(10000.0) / D))
    ang = pos * div
    pe = np.zeros((S, D), np.float32)
    pe[:, 0::2] = np.sin(ang)
    pe[:, 1::2] = np.cos(ang)
    return pe


def make_in_maps(x, Wq, Wfc, W1, b1, W2, b2, g1, be1, g2, be2):
    f32 = lambda a: np.ascontiguousarray(a, dtype=np.float32)
    bfc = lambda a: np.ascontiguousarray(np.asarray(f32(a), dtype="bfloat16"))
    xf = f32(x).reshape(S * H * W, D)
    pe = _pe_table()
    M2 = f32(Wfc).reshape(D, NH, DEP).sum(axis=1).T          # (64, 512)
    Mstk = np.concatenate([M2, M2], axis=0)                  # (128, 512)

    CF = np.zeros((128, NCF), np.float32)
    CF[:, O_EYE:O_EYE + 128] = np.eye(128, dtype=np.float32)
    b1p = f32(b1) + f32(W1) @ f32(be1)                       # be1 folded
    CF[:, O_B1P:O_B1P + 16] = b1p.reshape(16, 128).T

    CB = np.zeros((128, NCB), np.float32)
    CB[:, O_G1:O_G1 + D] = np.tile(f32(g1), (128, 1))
    CB[:, O_G2:O_G2 + D] = np.tile(f32(g2), (128, 1))
    CB[:, O_BE2:O_BE2 + D] = np.tile(f32(be2), (128, 1))
    CB[:, O_B1T:O_B1T + D] = np.tile(f32(b2) + f32(be1), (128, 1))
    CB[:, O_MST:O_MST + D] = Mstk
    CB[:, O_EYB:O_EYB + 128] = np.eye(128, dtype=np.float32)
    for jt in range(4):
        for p in range(128):
            CB[2 * jt + p // 64, O_E8 + jt * 128 + p] = 1.0

    # selT [8, 128]: selT[j, rr] = 1 iff rr//16 == j; peT per m-tile
    shared = dict(
        WqT=_round_f32r(Wq.T),
        W1T=bfc(f32(W1) * f32(g1)[None, :]).T.copy(),        # g1 folded
        W2T=bfc(f32(W2).T),
        CF=CF, CB=bfc(CB),
    )
    selT = np.zeros((8, 128), np.float32)
    for rr in range(128):
        selT[rr // 16, rr] = 1.0

    maps = []
    for k in range(NCORES):
        sl = xf[k * R:(k + 1) * R]
        m = dict(shared)
        slT = _round_f32r(sl.T)
        # xq layout: row-block i = m-pair (2i, 2i+1); columns (dt, mi, c)
        arr = slT.reshape(4, 128, 4, 2, 128)        # (t, p, i, mi, c)
        arr = arr.transpose(2, 1, 0, 3, 4)          # (i, p, t, mi, c)
        m["xT"] = np.ascontiguousarray(arr.reshape(512, 1024))
        # xRb[p, t*512+d] = x[t*128+p, d]
        m["xRb"] = bfc(sl.reshape(8, 128, D).transpose(1, 0, 2)
                       .reshape(128, 8 * D))
        cr8 = np.zeros((8, NCR8), np.float32)
        cr8[:, 0:128] = selT
        pe_loc = pe[k * 64:(k + 1) * 64]            # (64, 512)
        cr8[:, 128:] = _round_f32r(pe_loc.reshape(8, 8, D)
                                   .transpose(1, 0, 2).reshape(8, 8 * D))
        m["CR8"] = _round_f32r(cr8)
        maps.append(m)
    return maps


def kernel(x, Wq, Wfc, W1, b1, W2, b2, g1, be1, g2, be2, _results_hook=None,
           _trace=False, _tmpdir=None):
    if "nc" not in _cached:
        _cached["nc"] = build_nc()
    nc = _cached["nc"]
    in_maps = make_in_maps(x, Wq, Wfc, W1, b1, W2, b2, g1, be1, g2, be2)
    res = run_bass_kernel_spmd(nc, in_maps, list(range(NCORES)),
                               trace=_trace, tmpdir=_tmpdir)
    if _results_hook is not None:
        _results_hook(res)
    y = np.concatenate([res.results[k]["out"] for k in range(NCORES)], axis=0)
    return y.reshape(S, H, W, D)


# revision 26
# speedup vs baseline: 1.1259x; 1.0223x over previous
"""Trainium2 Bass kernel for nn_EncoderLayer_73315091743398.

The reference module's attention einsums ('hwink,hwijm->hwinm') sum their k/j
indices independently, so the whole attention block collapses to, per
(h,w)-chunk c and head i, over the flat q matrix qf = x@Wq.T + pe viewed as
(8192, 512) in raw (s,h,w) row order:

    u[s]  = sum_d qf[c*512+s, 64i+d]          (segment row sums)
    a     = softmax_s(u)
    v[d]  = sum_s a[s] * qf[c*512+s, 64i+d]
    row   = tile8(v) @ Wfc.T = v @ M,  M[d,:] = sum_b Wfc[:, 64b+d].T

and attn_out viewed (S,H,W,D) has row A[s'] = row_{c=s'//32, i=(s'%32)//4},
independent of (h,w).  Core k owns raw rows [k*1024,(k+1)*1024): these are
exactly attention chunks {2k, 2k+1} AND the residual/FFN rows for
s' in [64k, 64k+64), so the 8 cores run fully independent SPMD programs.

This version (vs the f32-heavy baseline):
  - pe is added inside the q-matmul PSUM group via a K=8 selector matmul
    (each 128-row tile uses 8 distinct pe rows, repeated 16x).
  - u is reduced directly from the q PSUM (exact f32; the peaked softmax
    needs it), while the SBUF copy of q used for v is bf16 via ScalarE.
  - residual adds z1/z2 are DVE tensor_adds producing bf16 tiles; LN stats
    and the normalize tensor_scalar then run in 2x bf16 mode.
  - o1 transpose for the FFN is a single SBUF->SBUF DMA-transpose per
    128-row tile (bf16 xbar path) instead of 4 PE transposes.
  - g1 is folded into W1, be1 into b1, (b2+be1) into the o1 tile, so the
    FFN stream is pure matmuls + one ScalarE relu per (half, ft).
  - per-column scale/bias (g2, be2) run on the otherwise-idle GpSimd.
"""

import math
import os
import sys
from contextlib import ExitStack

import numpy as np
import ml_dtypes  # noqa: F401  (registers bfloat16)

for _p in ("/opt/trn_rl_repo", "/root/.axon_site/_ro/trn_rl_repo"):
    if os.path.isdir(_p) and _p not in sys.path:
        sys.path.append(_p)

import concourse.bass as bass
import concourse.bacc as bacc
import concourse.mybir as mybir
import concourse.tile as tile
from concourse.bass_utils import run_bass_kernel_spmd

F32 = mybir.dt.float32
F32R = mybir.dt.float32r
BF16 = mybir.dt.bfloat16
AF = mybir.ActivationFunctionType
ALU = mybir.AluOpType
AX = mybir.AxisListType

S, H, W, D = 512, 4, 4, 512
NH, DEP, DFF = 8, 64, 2048
NCORES = 8
R = 1024          # rows per core of the flat (8192, 512) view
EPS = 1e-5

# CF (f32) column offsets: eye128, b1' packed [128, 16]
O_EYE, O_B1P = 0, 128
NCF = 144
# CB (bf16) column offsets
O_G1, O_G2, O_BE2, O_B1T, O_MST, O_E8, O_EYB = 0, 512, 1024, 1536, 2048, 2560, 3072
NCB = 3200
# CR8 (f32r, 8 partitions): selT [8,128], peT [8, 8*512]
NCR8 = 128 + 8 * D

_cached = {}


def build_nc():
    """Build the single-core SPMD Bass/Tile program (same program on all 8)."""
    nc = bacc.Bacc("TRN2", debug=False, target_bir_lowering=False)

    xT = nc.dram_tensor("xT", [D, R], F32R, kind="ExternalInput")
    xRb = nc.dram_tensor("xRb", [128, 8 * D], BF16, kind="ExternalInput")
    WqT = nc.dram_tensor("WqT", [D, D], F32R, kind="ExternalInput")
    W1T = nc.dram_tensor("W1T", [D, DFF], BF16, kind="ExternalInput")
    W2T = nc.dram_tensor("W2T", [DFF, D], BF16, kind="ExternalInput")
    CF = nc.dram_tensor("CF", [128, NCF], F32, kind="ExternalInput")
    CB = nc.dram_tensor("CB", [128, NCB], BF16, kind="ExternalInput")
    CR8 = nc.dram_tensor("CR8", [8, NCR8], F32R, kind="ExternalInput")
    out = nc.dram_tensor("out", [R, D], F32, kind="ExternalOutput")

    with ExitStack() as ctx:
        tc = ctx.enter_context(tile.TileContext(nc))
        cst = ctx.enter_context(tc.tile_pool(name="cst", bufs=1))
        xp = ctx.enter_context(tc.tile_pool(name="xp", bufs=1))
        qp = ctx.enter_context(tc.tile_pool(name="qp", bufs=1))
        hp = ctx.enter_context(tc.tile_pool(name="hp", bufs=1))
        wk = ctx.enter_context(tc.tile_pool(name="wk", bufs=2))
        ps = ctx.enter_context(tc.tile_pool(name="ps", bufs=1, space="PSUM"))

        # ---- loads: q-path inputs first (cr8/cf tiny, then wq/xq split into
        #      chunked DMAs so the first matmuls gate on minimal bytes);
        #      cb/xR/W1/W2 are gated later via a dummy write so their DMA
        #      doesn't steal HBM bandwidth from the critical q-path loads ----
        cr8 = cst.tile([8, NCR8], F32R, tag="cr8", name="cr8")
        nc.sync.dma_start(cr8[:], CR8[:])
        cf = cst.tile([128, NCF], F32, tag="cf", name="cf")
        nc.sync.dma_start(cf[:], CF[:])
        # wq chunk t + xq0 quarter t arrive together -> the zigzagged first
        # two q-tiles can start their dt=t matmuls as soon as each pair lands
        wq_all = cst.tile([128, 4 * D], F32R, tag="wq", name="wq_all")
        xq = [xp.tile([128, R], F32R, tag=f"dT{i}", name=f"xq{i}")
              for i in range(4)]
        for t in range(4):
            nc.sync.dma_start(wq_all[:, t * D:(t + 1) * D],
                              WqT[t * 128:(t + 1) * 128, :])
            nc.sync.dma_start(xq[0][:, t * 256:(t + 1) * 256],
                              xT[0:128, t * 256:(t + 1) * 256])
        for i in range(1, 4):
            nc.sync.dma_start(xq[i][:], xT[i * 128:(i + 1) * 128, :])

        # deferred loads (tiles declared now, DMA issued after a gate write)
        cb = cst.tile([128, NCB], BF16, tag="cb", name="cb")
        xr_all = xp.tile([128, 8 * D], BF16, tag="xr", name="xr_all")
        w1_all = cst.tile([128, 4 * DFF], BF16, tag="w1", name="w1_all")
        w2_all = cst.tile([128, 16 * D], BF16, tag="w2", name="w2_all")

        eye_sb = cf[:, O_EYE:O_EYE + 128]
        G1cb = cb[:, O_G1:O_G1 + D]
        G2cb = cb[:, O_G2:O_G2 + D]
        BE2cb = cb[:, O_BE2:O_BE2 + D]
        B1Tcb = cb[:, O_B1T:O_B1T + D]
        Mstcb = cb[:, O_MST:O_MST + D]
        EYBcb = cb[:, O_EYB:O_EYB + 128]
        epsT = cst.tile([128, 1], F32, tag="eps", name="epsT")
        nc.vector.memset(epsT[:], EPS)

        # ---- PE warm-up + ACT table preload, all during the DMA wait ----
        # HAM un-throttles the PE clock (1.2->2.4 GHz) only after ~3.4us of
        # sustained matmul activity; burn that in on junk data while the
        # inputs stream in.  Likewise the Exp/Sqrt activation tables load
        # lazily (~1.3us) -- force them now, off the critical softmax path.
        warm_sb = cst.tile([128, D], BF16, tag="wrm", name="warm_sb")
        nc.vector.memset(warm_sb[:], 0.0)
        wps = ps.tile([128, D], F32, tag="vc", bufs=2)
        for _ in range(16):
            nc.tensor.matmul(wps[:], warm_sb[:, 0:128], warm_sb[:],
                             start=True, stop=True)
        junk = cst.tile([128, 1], F32, tag="junk", name="junk")
        nc.scalar.activation(junk[:], epsT[:], AF.Exp)
        nc.scalar.activation(junk[:], epsT[:], AF.Sqrt)

        q_sb = [qp.tile([128, D], BF16, tag=f"q{m}", name=f"qsb{m}")
                for m in range(8)]
        nrm1 = [qp.tile([128, D], BF16, tag=f"n1{m}", name=f"nrm1_{m}")
                for m in range(8)]
        o1_sb = [qp.tile([128, D], BF16, tag=f"o1{m}", name=f"o1sb{m}")
                 for m in range(8)]
        # o1T_all[p, m*512 + t*128 + y] = nrm1[m][y, t*128 + p]
        o1T_all = qp.tile([128, 8 * D], BF16, tag="oT", name="o1T_all")
        uT = [qp.tile([8, D], F32, tag=f"uT{c}", name=f"uTsb{c}")
              for c in range(2)]

        qps_map = {}

        def q_mm(m):
            qps = ps.tile([128, D], F32, tag="mmQ", bufs=2)
            qps_map[m] = qps
            i, mi = divmod(m, 2)
            for dt in range(4):
                nc.tensor.matmul(
                    qps[:],
                    xq[i][:, dt * 256 + mi * 128:dt * 256 + (mi + 1) * 128],
                    wq_all[:, dt * D:(dt + 1) * D],
                    start=(dt == 0), stop=False)
            # += sel.T @ pe8  (adds the 8 distinct pe rows of this tile)
            nc.tensor.matmul(
                qps[:], cr8[:, 0:128],
                cr8[:, 128 + m * D:128 + (m + 1) * D],
                start=False, stop=True)

        def q_mm_pair(ma, mb):
            """Zigzag two q tiles' accumulation groups across two PSUM banks
            at dt granularity, so matmuls start as each wq/xq chunk lands."""
            qa = ps.tile([128, D], F32, tag="mmQ", bufs=2)
            qb = ps.tile([128, D], F32, tag="mmQ", bufs=2)
            qps_map[ma], qps_map[mb] = qa, qb
            for dt in range(4):
                for m, qp_ in ((ma, qa), (mb, qb)):
                    i, mi = divmod(m, 2)
                    nc.tensor.matmul(
                        qp_[:],
                        xq[i][:, dt * 256 + mi * 128:dt * 256 + (mi + 1) * 128],
                        wq_all[:, dt * D:(dt + 1) * D],
                        start=(dt == 0), stop=False)
            for m, qp_ in ((ma, qa), (mb, qb)):
                nc.tensor.matmul(
                    qp_[:], cr8[:, 0:128],
                    cr8[:, 128 + m * D:128 + (m + 1) * D],
                    start=False, stop=True)

        def q_fin(m, copy=True):
            qps = qps_map[m]
            useg = wk.tile([128, 8], F32, tag="useg")
            nc.vector.tensor_reduce(
                useg[:], qps[:].rearrange("p (h d) -> p h d", h=8),
                axis=AX.X, op=ALU.add)
            if copy:
                q_copy(m)
            utp = ps.tile([8, 128], F32, tag="vc", bufs=2)
            nc.tensor.transpose(utp[:], useg[:], eye_sb)
            c, st = divmod(m, 4)
            nc.vector.tensor_copy(uT[c][:, st * 128:(st + 1) * 128], utp[:])

        def q_copy(m):
            nc.scalar.copy(q_sb[m][:], qps_map[m][:])   # f32 psum -> bf16 sbuf

        def warm_fill(n):
            """Dummy matmuls to keep the PE HAM-warm across dependency waits."""
            wtile = ps.tile([128, D], F32, tag="vc", bufs=2)
            for _ in range(n):
                nc.tensor.matmul(wtile[:], warm_sb[:, 0:128], warm_sb[:],
                                 start=True, stop=True)

        def attn_softmax(c):
            mx = wk.tile([8, 1], F32, tag="mx")
            nc.vector.tensor_reduce(mx[:], uT[c][:], axis=AX.X, op=ALU.max)
            nmx = wk.tile([8, 1], F32, tag="nmx")
            nc.vector.tensor_scalar_mul(nmx[:], mx[:], -1.0)
            ex = wk.tile([8, D], F32, tag=f"ex{c}", bufs=1)
            ssum = wk.tile([8, 1], F32, tag="esum")
            nc.scalar.activation(ex[:], uT[c][:], AF.Exp, bias=nmx[:, :],
                                 accum_out=ssum[:])
            rcp = wk.tile([8, 1], F32, tag="ercp")
            nc.vector.reciprocal(rcp[:], ssum[:])
            nc.vector.tensor_scalar_mul(ex[:], ex[:], rcp[:])
            return ex

        def attn_prods(c, ex):
            aTss = []
            for st in range(4):
                atp = ps.tile([128, 8], F32, tag="vc", bufs=2)
                nc.tensor.transpose(atp[:], ex[:, st * 128:(st + 1) * 128],
                                    eye_sb[:8, :8])
                aTs = wk.tile([128, 8], BF16, tag=f"aT{c}{st}", bufs=1)
                nc.vector.tensor_copy(aTs[:], atp[:])
                aTss.append(aTs)
            return aTss

        def attn_la(c, aTss):
            vm = wk.tile([128, 8], BF16, tag=f"vm{c}", bufs=1)
            nc.vector.memset(vm[:], 0.0)
            for jt in range(4):
                # wsum[p, i] = sum_s q[s, 128*jt+p] * a_i[s]
                wsum = ps.tile([128, 8], F32, tag="vc", bufs=2)
                for st in range(4):
                    nc.tensor.matmul(
                        wsum[:], q_sb[c * 4 + st][:, jt * 128:(jt + 1) * 128],
                        aTss[st][:], start=(st == 0), stop=(st == 3))
                nc.vector.tensor_copy(vm[0:64, 2 * jt:2 * jt + 1],
                                      wsum[0:64, 2 * jt:2 * jt + 1])
                nc.vector.tensor_copy(vm[64:128, 2 * jt + 1:2 * jt + 2],
                                      wsum[64:128, 2 * jt + 1:2 * jt + 2])
            lap = ps.tile([8, D], F32, tag="vc", bufs=2)
            nc.tensor.matmul(lap[:], vm[:], Mstcb, start=True, stop=True)
            las = wk.tile([8, D], BF16, tag=f"las{c}", bufs=1)
            nc.vector.tensor_copy(las[:], lap[:])
            return las

        def attn_resid(c, las, jt):
            m = c * 4 + jt
            bcp = ps.tile([128, D], F32, tag="mmB", bufs=2)
            nc.tensor.matmul(bcp[:],
                             cb[0:8, O_E8 + jt * 128:O_E8 + (jt + 1) * 128],
                             las[:], start=True, stop=False)
            # z1 += x residual on the PE (keeps the DVE chain short)
            nc.tensor.matmul(bcp[:], EYBcb, xr_all[:, m * D:(m + 1) * D],
                             start=False, stop=True)
            st6 = wk.tile([128, 6], F32, tag="ls")
            nc.vector.bn_stats(st6[:], bcp[:])
            mv = wk.tile([128, 2], F32, tag="lm")
            nc.vector.bn_aggr(mv[:], st6[:])
            sd = wk.tile([128, 1], F32, tag="lsd")
            nc.scalar.activation(sd[:], mv[:, 1:2], AF.Sqrt, bias=epsT[:, :])
            rsd = wk.tile([128, 1], F32, tag="lr")
            nc.vector.reciprocal(rsd[:], sd[:])
            if c == 0:
                # c0 gates the first FFN half: normalize on ScalarE
                # (out = z1*rsd + (-mu*rsd)), keeping the DVE free
                nmr = wk.tile([128, 1], F32, tag="nmr")
                nc.vector.tensor_scalar(nmr[:], mv[:, 0:1], rsd[:], -1.0,
                                        op0=ALU.mult, op1=ALU.mult)
                nc.scalar.activation(nrm1[m][:], bcp[:], AF.Identity,
                                     bias=nmr[:, :], scale=rsd[:, :])
            else:
                nc.vector.tensor_scalar(nrm1[m][:], bcp[:], mv[:, 0:1], rsd[:],
                                        op0=ALU.subtract, op1=ALU.mult)
            # o1 = nrm1*g1 + (b2+be1)   (GpSimd; consumed by the z2 add)
            nc.gpsimd.tensor_mul(o1_sb[m][:], nrm1[m][:], G1cb)
            nc.gpsimd.tensor_add(o1_sb[m][:], o1_sb[m][:], B1Tcb)
            # contiguous xbar dest: o1T_all[p, m*512 + t*128 + y] = nrm1[y, t*128+p]
            # alternate HWDGE queues (sync/scalar) so the ~1.2us descriptor
            # generation of consecutive transposes overlaps
            eng = nc.sync if m % 2 == 0 else nc.scalar
            eng.dma_start_transpose(
                o1T_all[:, m * D:(m + 1) * D].rearrange("p (t y) -> p t y", t=4),
                nrm1[m][:])

        h1map = {}

        # strided rhs view: oTr[p, m, t, y] = o1T_all[p, m*512 + t*128 + y]
        oTr = o1T_all[:].rearrange("p (m t y) -> p m t y", m=8, t=4)

        def ffn_h1(h, ft, split=False):
            p1 = ps.tile([128, D], F32, tag="mmA", bufs=2)
            if split:
                # two N=256 halves: the first needs only m-tiles (h*4, h*4+1),
                # so it can start before the later o1T transposes land.
                # half 1 uses start=False on untouched PSUM (has_written unset
                # -> overwrite), so half 0's values are never clobbered.
                for half in range(2):
                    for dt in range(4):
                        nc.tensor.matmul(
                            p1[:, half * 256:(half + 1) * 256],
                            w1_all[:, dt * DFF + ft * 128:
                                   dt * DFF + (ft + 1) * 128],
                            oTr[:, h * 4 + 2 * half:h * 4 + 2 * half + 2,
                                dt, :],
                            start=(half == 0 and dt == 0), stop=(dt == 3),
                            skip_group_check=True)
            else:
                for dt in range(4):
                    nc.tensor.matmul(
                        p1[:],
                        w1_all[:, dt * DFF + ft * 128:dt * DFF + (ft + 1) * 128],
                        oTr[:, h * 4:(h + 1) * 4, dt, :],
                        start=(dt == 0), stop=(dt == 3))
            h1t = hp.tile([128, D], BF16, tag=f"h1_{ft}", bufs=2,
                          name=f"h1_{h}_{ft}")
            nc.scalar.activation(h1t[:], p1[:], AF.Relu,
                                 bias=cf[:, O_B1P + ft:O_B1P + ft + 1])
            h1map[(h, ft)] = h1t

        def ffn_rm(m):
            h, rm = divmod(m, 4)
            tail = m >= 6
            p2 = ps.tile([128, D], F32, tag="mmB", bufs=2)
            for ft in range(16):
                nc.tensor.matmul(
                    p2[:], h1map[(h, ft)][:, rm * 128:(rm + 1) * 128],
                    w2_all[:, ft * D:(ft + 1) * D],
                    start=(ft == 0), stop=(ft == 15 and not tail))
            if tail:
                # z2 += o1 on the (tail-idle) PE; LN2 reads PSUM directly,
                # shortening the post-matmul serial chain of the last tiles
                nc.tensor.matmul(p2[:], EYBcb, o1_sb[m][:],
                                 start=False, stop=True)
                z2 = p2
            else:
                z2t = wk.tile([128, D], BF16, tag="z2")
                nc.vector.tensor_add(z2t[:], p2[:], o1_sb[m][:])
                z2 = z2t
            st6 = wk.tile([128, 6], F32, tag="ls")
            nc.vector.bn_stats(st6[:], z2[:])
            mv = wk.tile([128, 2], F32, tag="lm")
            nc.vector.bn_aggr(mv[:], st6[:])
            sd = wk.tile([128, 1], F32, tag="lsd")
            nc.scalar.activation(sd[:], mv[:, 1:2], AF.Sqrt, bias=epsT[:, :])
            rsd = wk.tile([128, 1], F32, tag="lr")
            nc.vector.reciprocal(rsd[:], sd[:])
            nrm2 = wk.tile([128, D], BF16, tag="n2")
            nc.vector.tensor_scalar(nrm2[:], z2[:], mv[:, 0:1], rsd[:],
                                    op0=ALU.subtract, op1=ALU.mult)
            tg = wk.tile([128, D], BF16, tag="tg")
            yt = wk.tile([128, D], F32, tag="yt")
            if tail:
                nc.vector.tensor_mul(tg[:], nrm2[:], G2cb)
                nc.vector.tensor_add(yt[:], tg[:], BE2cb)
            else:
                nc.gpsimd.tensor_mul(tg[:], nrm2[:], G2cb)
                nc.gpsimd.tensor_add(yt[:], tg[:], BE2cb)
            nc.sync.dma_start(out[m * 128:(m + 1) * 128, :], yt[:])

        def gate_dma(tile_t, tile_ap, dram):
            """Order a big load after earlier compute: a 1-elem dummy write
            creates a WAR dependency the DMA must wait on."""
            nc.vector.memset(tile_t[0:1, 0:1], 0.0)
            nc.sync.dma_start(tile_ap, dram)

        # ---------------- schedule ----------------
        # ungated bulk loads last in issue order (cb, xr); W1/W2 gated
        nc.sync.dma_start(cb[:], CB[:])
        nc.sync.dma_start(xr_all[:, 0:4 * D], xRb[:, 0:4 * D])
        nc.sync.dma_start(xr_all[:, 4 * D:], xRb[:, 4 * D:])
        q_mm_pair(0, 1)
        q_fin(0)
        q_fin(1)
        q_mm(2)
        q_fin(2)
        q_mm(3)
        q_fin(3)
        gate_dma(w1_all, w1_all[:].rearrange("p (t j) -> p t j", t=4),
                 W1T.rearrange("(t p) j -> p t j", p=128))
        a0 = attn_softmax(0)
        aT0 = attn_prods(0, a0)
        gate_dma(w2_all, w2_all[:].rearrange("p (t j) -> p t j", t=16),
                 W2T.rearrange("(t p) j -> p t j", p=128))
        q_mm(4)
        q_fin(4, copy=False)
        las0 = attn_la(0, aT0)
        q_mm(5)
        q_fin(5, copy=False)
        attn_resid(0, las0, 0)
        attn_resid(0, las0, 1)
        q_mm(6)
        q_mm(7)
        warm_fill(4)
        q_fin(6, copy=False)
        q_fin(7, copy=False)
        attn_resid(0, las0, 2)
        attn_resid(0, las0, 3)
        warm_fill(4)
        q_copy(4)
        q_copy(5)
        a1 = attn_softmax(1)
        aT1 = attn_prods(1, a1)
        q_copy(6)
        q_copy(7)
        warm_fill(4)
        las1 = attn_la(1, aT1)
        attn_resid(1, las1, 0)
        attn_resid(1, las1, 1)
        for ft in range(8):
            ffn_h1(0, ft, split=True)
        attn_resid(1, las1, 2)
        for ft in range(8, 12):
            ffn_h1(0, ft, split=True)
        attn_resid(1, las1, 3)
        for ft in range(12, 16):
            ffn_h1(0, ft, split=True)
        ffn_rm(0)
        for ft in range(0, 4):
            ffn_h1(1, ft)
        ffn_rm(1)
        for ft in range(4, 8):
            ffn_h1(1, ft)
        ffn_rm(2)
        for ft in range(8, 12):
            ffn_h1(1, ft)
        ffn_rm(3)
        for ft in range(12, 16):
            ffn_h1(1, ft)
        for m in range(4, 8):
            ffn_rm(m)

    nc.compile()
    return nc


def _round_f32r(a):
    b = np.ascontiguousarray(a, dtype=np.float32).view(np.uint32)
    out = (b + 0x7FF + ((b >> 12) & 1)) & np.uint32(0xFFFFF000)
    return out.view(np.float32)


def _pe_table():
    pos = np.arange(S, dtype=np.float32)[:, None]
    div = np.exp(np.arange(0, D, 2, dtype=np.float32) * (-math.log(10000.0) / D))
    ang = pos * div
    pe = np.zeros((S, D), np.float32)
    pe[:, 0::2] = np.sin(ang)
    pe[:, 1::2] = np.cos(ang)
    return pe


def make_in_maps(x, Wq, Wfc, W1, b1, W2, b2, g1, be1, g2, be2):
    f32 = lambda a: np.ascontiguousarray(a, dtype=np.float32)
    bfc = lambda a: np.ascontiguousarray(np.asarray(f32(a), dtype="bfloat16"))
    xf = f32(x).reshape(S * H * W, D)
    pe = _pe_table()
    M2 = f32(Wfc).reshape(D, NH, DEP).sum(axis=1).T          # (64, 512)
    Mstk = np.concatenate([M2, M2], axis=0)                  # (128, 512)

    CF = np.zeros((128, NCF), np.float32)
    CF[:, O_EYE:O_EYE + 128] = np.eye(128, dtype=np.float32)
    b1p = f32(b1) + f32(W1) @ f32(be1)                       # be1 folded
    CF[:, O_B1P:O_B1P + 16] = b1p.reshape(16, 128).T

    CB = np.zeros((128, NCB), np.float32)
    CB[:, O_G1:O_G1 + D] = np.tile(f32(g1), (128, 1))
    CB[:, O_G2:O_G2 + D] = np.tile(f32(g2), (128, 1))
    CB[:, O_BE2:O_BE2 + D] = np.tile(f32(be2), (128, 1))
    CB[:, O_B1T:O_B1T + D] = np.tile(f32(b2) + f32(be1), (128, 1))
    CB[:, O_MST:O_MST + D] = Mstk
    CB[:, O_EYB:O_EYB + 128] = np.eye(128, dtype=np.float32)
    for jt in range(4):
        for p in range(128):
            CB[2 * jt + p // 64, O_E8 + jt * 128 + p] = 1.0

    # selT [8, 128]: selT[j, rr] = 1 iff rr//16 == j; peT per m-tile
    shared = dict(
        WqT=_round_f32r(Wq.T),
        W1T=bfc(f32(W1) * f32(g1)[None, :]).T.copy(),        # g1 folded
        W2T=bfc(f32(W2).T),
        CF=CF, CB=bfc(CB),
    )
    selT = np.zeros((8, 128), np.float32)
    for rr in range(128):
        selT[rr // 16, rr] = 1.0

    maps = []
    for k in range(NCORES):
        sl = xf[k * R:(k + 1) * R]
        m = dict(shared)
        slT = _round_f32r(sl.T)
        # xq layout: row-block i = m-pair (2i, 2i+1); columns (dt, mi, c)
        arr = slT.reshape(4, 128, 4, 2, 128)        # (t, p, i, mi, c)
        arr = arr.transpose(2, 1, 0, 3, 4)          # (i, p, t, mi, c)
        m["xT"] = np.ascontiguousarray(arr.reshape(512, 1024))
        # xRb[p, t*512+d] = x[t*128+p, d]
        m["xRb"] = bfc(sl.reshape(8, 128, D).transpose(1, 0, 2)
                       .reshape(128, 8 * D))
        cr8 = np.zeros((8, NCR8), np.float32)
        cr8[:, 0:128] = selT
        pe_loc = pe[k * 64:(k + 1) * 64]            # (64, 512)
        cr8[:, 128:] = _round_f32r(pe_loc.reshape(8, 8, D)
                                   .transpose(1, 0, 2).reshape(8, 8 * D))
        m["CR8"] = _round_f32r(cr8)
        maps.append(m)
    return maps


def kernel(x, Wq, Wfc, W1, b1, W2, b2, g1, be1, g2, be2, _results_hook=None,
           _trace=False, _tmpdir=None):
    if "nc" not in _cached:
        _cached["nc"] = build_nc()
    nc = _cached["nc"]
    in_maps = make_in_maps(x, Wq, Wfc, W1, b1, W2, b2, g1, be1, g2, be2)
    res = run_bass_kernel_spmd(nc, in_maps, list(range(NCORES)),
                               trace=_trace, tmpdir=_tmpdir)
    if _results_hook is not None:
        _results_hook(res)
    y = np.concatenate([res.results[k]["out"] for k in range(NCORES)], axis=0)
    return y.reshape(S, H, W, D)


# revision 34
# speedup vs baseline: 1.2031x; 1.0686x over previous
"""Trainium2 Bass kernel for nn_EncoderLayer_73315091743398.

The reference module's attention einsums ('hwink,hwijm->hwinm') sum their k/j
indices independently, so the whole attention block collapses to, per
(h,w)-chunk c and head i, over the flat q matrix qf = x@Wq.T + pe viewed as
(8192, 512) in raw (s,h,w) row order:

    u[s]  = sum_d qf[c*512+s, 64i+d]          (segment row sums)
    a     = softmax_s(u)
    v[d]  = sum_s a[s] * qf[c*512+s, 64i+d]
    row   = tile8(v) @ Wfc.T = v @ M,  M[d,:] = sum_b Wfc[:, 64b+d].T

and attn_out viewed (S,H,W,D) has row A[s'] = row_{c=s'//32, i=(s'%32)//4},
independent of (h,w).  Core k owns raw rows [k*1024,(k+1)*1024): these are
exactly attention chunks {2k, 2k+1} AND the residual/FFN rows for
s' in [64k, 64k+64), so the 8 cores run fully independent SPMD programs.

This version (vs the f32-heavy baseline):
  - pe is added inside the q-matmul PSUM group via a K=8 selector matmul
    (each 128-row tile uses 8 distinct pe rows, repeated 16x).
  - u is reduced directly from the q PSUM (exact f32; the peaked softmax
    needs it), while the SBUF copy of q used for v is bf16 via ScalarE.
  - residual adds z1/z2 are DVE tensor_adds producing bf16 tiles; LN stats
    and the normalize tensor_scalar then run in 2x bf16 mode.
  - o1 transpose for the FFN is a single SBUF->SBUF DMA-transpose per
    128-row tile (bf16 xbar path) instead of 4 PE transposes.
  - g1 is folded into W1, be1 into b1, (b2+be1) into the o1 tile, so the
    FFN stream is pure matmuls + one ScalarE relu per (half, ft).
  - per-column scale/bias (g2, be2) run on the otherwise-idle GpSimd.
"""

import math
import os
import sys
from contextlib import ExitStack

import numpy as np
import ml_dtypes  # noqa: F401  (registers bfloat16)

for _p in ("/opt/trn_rl_repo", "/root/.axon_site/_ro/trn_rl_repo"):
    if os.path.isdir(_p) and _p not in sys.path:
        sys.path.append(_p)

import concourse.bass as bass
import concourse.bacc as bacc
import concourse.mybir as mybir
import concourse.tile as tile
from concourse.bass_utils import run_bass_kernel_spmd

F32 = mybir.dt.float32
F32R = mybir.dt.float32r
F16 = mybir.dt.float16
BF16 = mybir.dt.bfloat16
AF = mybir.ActivationFunctionType
ALU = mybir.AluOpType
AX = mybir.AxisListType

S, H, W, D = 512, 4, 4, 512
NH, DEP, DFF = 8, 64, 2048
NCORES = 8
R = 1024          # rows per core of the flat (8192, 512) view
EPS = 1e-5

# CF (f32) column offsets: eye128, b1' packed [128, 16]
O_EYE, O_B1P = 0, 128
NCF = 144
# CB (bf16): eye128 + Mst
O_EYB, O_MST = 0, 128
NCB = 640
# CS8 (bf16, 8 partitions): E8 selector rows + single rows of g1/g2/be2/(b2+be1)
O_E8, O_G1R, O_G2R, O_BE2R, O_B1TR = 0, 512, 1024, 1536, 2048
NCS = 2560
# CR8 (fp16, 8 partitions): selT [8,128], peT [8, 8*512]
NCR8 = 128 + 8 * D

_cached = {}


def build_nc():
    """Build the single-core SPMD Bass/Tile program (same program on all 8)."""
    nc = bacc.Bacc("TRN2", debug=False, target_bir_lowering=False)

    xT = nc.dram_tensor("xT", [D, R], F16, kind="ExternalInput")
    xRb = nc.dram_tensor("xRb", [128, 8 * D], BF16, kind="ExternalInput")
    WqT = nc.dram_tensor("WqT", [D, D], F16, kind="ExternalInput")
    W1T = nc.dram_tensor("W1T", [D, DFF], BF16, kind="ExternalInput")
    W2T = nc.dram_tensor("W2T", [DFF, D], BF16, kind="ExternalInput")
    CF = nc.dram_tensor("CF", [128, NCF], F32, kind="ExternalInput")
    CB = nc.dram_tensor("CB", [128, NCB], BF16, kind="ExternalInput")
    CS8 = nc.dram_tensor("CS8", [8, NCS], BF16, kind="ExternalInput")
    CR8 = nc.dram_tensor("CR8", [8, NCR8], F16, kind="ExternalInput")
    out = nc.dram_tensor("out", [R, D], F32, kind="ExternalOutput")

    with ExitStack() as ctx:
        tc = ctx.enter_context(tile.TileContext(nc))
        cst = ctx.enter_context(tc.tile_pool(name="cst", bufs=1))
        xp = ctx.enter_context(tc.tile_pool(name="xp", bufs=1))
        qp = ctx.enter_context(tc.tile_pool(name="qp", bufs=1))
        hp = ctx.enter_context(tc.tile_pool(name="hp", bufs=1))
        wk = ctx.enter_context(tc.tile_pool(name="wk", bufs=2))
        ps = ctx.enter_context(tc.tile_pool(name="ps", bufs=1, space="PSUM"))

        # ---- loads: q-path inputs first (cr8/cf tiny, then wq/xq split into
        #      chunked DMAs so the first matmuls gate on minimal bytes);
        #      cb/xR/W1/W2 are gated later via a dummy write so their DMA
        #      doesn't steal HBM bandwidth from the critical q-path loads ----
        cr8 = cst.tile([8, NCR8], F16, tag="cr8", name="cr8")
        nc.sync.dma_start(cr8[:], CR8[:])
        cf = cst.tile([128, NCF], F32, tag="cf", name="cf")
        nc.sync.dma_start(cf[:], CF[:])
        # wq chunk t + xq0 quarter t arrive together -> the zigzagged first
        # two q-tiles can start their dt=t matmuls as soon as each pair lands
        wq_all = cst.tile([128, 4 * D], F16, tag="wq", name="wq_all")
        xq = [xp.tile([128, R], F16, tag=f"dT{i}", name=f"xq{i}")
              for i in range(4)]
        for t in range(4):
            nc.sync.dma_start(wq_all[:, t * D:(t + 1) * D],
                              WqT[t * 128:(t + 1) * 128, :])
            nc.sync.dma_start(xq[0][:, t * 256:(t + 1) * 256],
                              xT[0:128, t * 256:(t + 1) * 256])
        for i in range(1, 4):
            nc.sync.dma_start(xq[i][:], xT[i * 128:(i + 1) * 128, :])
        w1_all = cst.tile([128, 4 * DFF], BF16, tag="w1", name="w1_all")
        nc.sync.dma_start(w1_all[:].rearrange("p (t j) -> p t j", t=4),
                          W1T.rearrange("(t p) j -> p t j", p=128))
        cb = cst.tile([128, NCB], BF16, tag="cb", name="cb")
        nc.sync.dma_start(cb[:], CB[:])
        cs8 = cst.tile([8, NCS], BF16, tag="cs8", name="cs8")
        nc.sync.dma_start(cs8[:], CS8[:])
        xr_all = xp.tile([128, 8 * D], BF16, tag="xr", name="xr_all")
        nc.sync.dma_start(xr_all[:, 0:4 * D], xRb[:, 0:4 * D])
        nc.sync.dma_start(xr_all[:, 4 * D:], xRb[:, 4 * D:])
        w2_all = cst.tile([128, 16 * D], BF16, tag="w2", name="w2_all")
        nc.sync.dma_start(w2_all[:].rearrange("p (t j) -> p t j", t=16),
                          W2T.rearrange("(t p) j -> p t j", p=128))

        eye_sb = cf[:, O_EYE:O_EYE + 128]
        Mstcb = cb[:, O_MST:O_MST + D]
        EYBcb = cb[:, O_EYB:O_EYB + 128]
        epsT = cst.tile([128, 1], F32, tag="eps", name="epsT")
        nc.vector.memset(epsT[:], EPS)

        # broadcast per-column vectors (4KB of DMA instead of 0.5MB)
        G1cb = cst.tile([128, D], BF16, tag="g1t", name="g1t")
        G2cb = cst.tile([128, D], BF16, tag="g2t", name="g2t")
        BE2cb = cst.tile([128, D], BF16, tag="be2t", name="be2t")
        B1Tcb = cst.tile([128, D], BF16, tag="b1tt", name="b1tt")
        for bt, off in ((G1cb, O_G1R), (G2cb, O_G2R),
                        (BE2cb, O_BE2R), (B1Tcb, O_B1TR)):
            nc.gpsimd.partition_broadcast(bt[:], cs8[0:1, off:off + D])

        # ---- PE warm-up + ACT table preload, all during the DMA wait ----
        # HAM un-throttles the PE clock (1.2->2.4 GHz) only after ~3.4us of
        # sustained matmul activity; burn that in on junk data while the
        # inputs stream in.  Likewise the Exp/Sqrt activation tables load
        # lazily (~1.3us) -- force them now, off the critical softmax path.
        warm_sb = cst.tile([128, D], BF16, tag="wrm", name="warm_sb")
        nc.vector.memset(warm_sb[:], 0.0)
        wps = ps.tile([128, D], F32, tag="vc", bufs=2)
        for _ in range(16):
            nc.tensor.matmul(wps[:], warm_sb[:, 0:128], warm_sb[:],
                             start=True, stop=True)
        junk = cst.tile([128, 1], F32, tag="junk", name="junk")
        nc.scalar.activation(junk[:], epsT[:], AF.Exp)
        nc.scalar.activation(junk[:], epsT[:], AF.Sqrt)

        q_sb = [qp.tile([128, D], BF16, tag=f"q{m}", name=f"qsb{m}")
                for m in range(8)]
        nrm1 = [qp.tile([128, D], BF16, tag=f"n1{m}", name=f"nrm1_{m}")
                for m in range(8)]
        o1_sb = [qp.tile([128, D], BF16, tag=f"o1{m}", name=f"o1sb{m}")
                 for m in range(8)]
        # o1T_all[p, m*512 + t*128 + y] = nrm1[m][y, t*128 + p]
        o1T_all = qp.tile([128, 8 * D], BF16, tag="oT", name="o1T_all")
        uT = [qp.tile([8, D], F32, tag=f"uT{c}", name=f"uTsb{c}")
              for c in range(2)]

        qps_map = {}

        def q_mm(m):
            qps = ps.tile([128, D], F32, tag="mmQ", bufs=2)
            qps_map[m] = qps
            i, mi = divmod(m, 2)
            for dt in range(4):
                nc.tensor.matmul(
                    qps[:],
                    xq[i][:, dt * 256 + mi * 128:dt * 256 + (mi + 1) * 128],
                    wq_all[:, dt * D:(dt + 1) * D],
                    start=(dt == 0), stop=False)
            # += sel.T @ pe8  (adds the 8 distinct pe rows of this tile)
            nc.tensor.matmul(
                qps[:], cr8[:, 0:128],
                cr8[:, 128 + m * D:128 + (m + 1) * D],
                start=False, stop=True)

        def q_mm_pair(ma, mb):
            """Zigzag two q tiles' accumulation groups across two PSUM banks
            at dt granularity, so matmuls start as each wq/xq chunk lands."""
            qa = ps.tile([128, D], F32, tag="mmQ", bufs=2)
            qb = ps.tile([128, D], F32, tag="mmQ", bufs=2)
            qps_map[ma], qps_map[mb] = qa, qb
            for dt in range(4):
                for m, qp_ in ((ma, qa), (mb, qb)):
                    i, mi = divmod(m, 2)
                    nc.tensor.matmul(
                        qp_[:],
                        xq[i][:, dt * 256 + mi * 128:dt * 256 + (mi + 1) * 128],
                        wq_all[:, dt * D:(dt + 1) * D],
                        start=(dt == 0), stop=False)
            for m, qp_ in ((ma, qa), (mb, qb)):
                nc.tensor.matmul(
                    qp_[:], cr8[:, 0:128],
                    cr8[:, 128 + m * D:128 + (m + 1) * D],
                    start=False, stop=True)

        def q_fin(m, copy=True):
            qps = qps_map[m]
            useg = wk.tile([128, 8], F32, tag="useg")
            nc.vector.tensor_reduce(
                useg[:], qps[:].rearrange("p (h d) -> p h d", h=8),
                axis=AX.X, op=ALU.add)
            if copy:
                q_copy(m)
            utp = ps.tile([8, 128], F32, tag="vc", bufs=2)
            nc.tensor.transpose(utp[:], useg[:], eye_sb)
            c, st = divmod(m, 4)
            nc.vector.tensor_copy(uT[c][:, st * 128:(st + 1) * 128], utp[:])

        def q_copy(m):
            nc.scalar.copy(q_sb[m][:], qps_map[m][:])   # f32 psum -> bf16 sbuf

        def warm_fill(n):
            """Dummy matmuls to keep the PE HAM-warm across dependency waits."""
            wtile = ps.tile([128, D], F32, tag="vc", bufs=2)
            for _ in range(n):
                nc.tensor.matmul(wtile[:], warm_sb[:, 0:128], warm_sb[:],
                                 start=True, stop=True)

        def attn_softmax(c):
            mx = wk.tile([8, 1], F32, tag="mx")
            nc.vector.tensor_reduce(mx[:], uT[c][:], axis=AX.X, op=ALU.max)
            nmx = wk.tile([8, 1], F32, tag="nmx")
            nc.vector.tensor_scalar_mul(nmx[:], mx[:], -1.0)
            ex = wk.tile([8, D], F32, tag=f"ex{c}", bufs=1)
            ssum = wk.tile([8, 1], F32, tag="esum")
            nc.scalar.activation(ex[:], uT[c][:], AF.Exp, bias=nmx[:, :],
                                 accum_out=ssum[:])
            rcp = wk.tile([8, 1], F32, tag="ercp")
            nc.vector.reciprocal(rcp[:], ssum[:])
            nc.vector.tensor_scalar_mul(ex[:], ex[:], rcp[:])
            return ex

        def attn_prods(c, ex):
            aTss = []
            for st in range(4):
                atp = ps.tile([128, 8], F32, tag="vc", bufs=2)
                nc.tensor.transpose(atp[:], ex[:, st * 128:(st + 1) * 128],
                                    eye_sb[:8, :8])
                aTs = wk.tile([128, 8], BF16, tag=f"aT{c}{st}", bufs=1)
                nc.vector.tensor_copy(aTs[:], atp[:])
                aTss.append(aTs)
            return aTss

        def attn_la(c, aTss):
            vm = wk.tile([128, 8], BF16, tag=f"vm{c}", bufs=1)
            nc.vector.memset(vm[:], 0.0)
            for jt in range(4):
                # wsum[p, i] = sum_s q[s, 128*jt+p] * a_i[s]
                wsum = ps.tile([128, 8], F32, tag="vc", bufs=2)
                for st in range(4):
                    nc.tensor.matmul(
                        wsum[:], q_sb[c * 4 + st][:, jt * 128:(jt + 1) * 128],
                        aTss[st][:], start=(st == 0), stop=(st == 3))
                nc.vector.tensor_copy(vm[0:64, 2 * jt:2 * jt + 1],
                                      wsum[0:64, 2 * jt:2 * jt + 1])
                nc.vector.tensor_copy(vm[64:128, 2 * jt + 1:2 * jt + 2],
                                      wsum[64:128, 2 * jt + 1:2 * jt + 2])
            lap = ps.tile([8, D], F32, tag="vc", bufs=2)
            nc.tensor.matmul(lap[:], vm[:], Mstcb, start=True, stop=True)
            las = wk.tile([8, D], BF16, tag=f"las{c}", bufs=1)
            nc.vector.tensor_copy(las[:], lap[:])
            return las

        def attn_resid(c, las, jt):
            m = c * 4 + jt
            bcp = ps.tile([128, D], F32, tag="mmB", bufs=2)
            nc.tensor.matmul(bcp[:],
                             cs8[0:8, O_E8 + jt * 128:O_E8 + (jt + 1) * 128],
                             las[:], start=True, stop=False)
            # z1 += x residual on the PE (keeps the DVE chain short)
            nc.tensor.matmul(bcp[:], EYBcb, xr_all[:, m * D:(m + 1) * D],
                             start=False, stop=True)
            st6 = wk.tile([128, 6], F32, tag="ls")
            nc.vector.bn_stats(st6[:], bcp[:])
            mv = wk.tile([128, 2], F32, tag="lm")
            nc.vector.bn_aggr(mv[:], st6[:])
            sd = wk.tile([128, 1], F32, tag="lsd")
            nc.scalar.activation(sd[:], mv[:, 1:2], AF.Sqrt, bias=epsT[:, :])
            rsd = wk.tile([128, 1], F32, tag="lr")
            nc.vector.reciprocal(rsd[:], sd[:])
            if c == 0:
                # c0 gates the first FFN half: normalize on ScalarE
                # (out = z1*rsd + (-mu*rsd)), keeping the DVE free
                nmr = wk.tile([128, 1], F32, tag="nmr")
                nc.vector.tensor_scalar(nmr[:], mv[:, 0:1], rsd[:], -1.0,
                                        op0=ALU.mult, op1=ALU.mult)
                nc.scalar.activation(nrm1[m][:], bcp[:], AF.Identity,
                                     bias=nmr[:, :], scale=rsd[:, :])
            else:
                nc.vector.tensor_scalar(nrm1[m][:], bcp[:], mv[:, 0:1], rsd[:],
                                        op0=ALU.subtract, op1=ALU.mult)
            # o1 = nrm1*g1 + (b2+be1)   (GpSimd; consumed by the z2 add)
            nc.gpsimd.tensor_mul(o1_sb[m][:], nrm1[m][:], G1cb[:])
            nc.gpsimd.tensor_add(o1_sb[m][:], o1_sb[m][:], B1Tcb[:])
            # contiguous xbar dest: o1T_all[p, m*512 + t*128 + y] = nrm1[y, t*128+p]
            # alternate HWDGE queues (sync/scalar) so the ~1.2us descriptor
            # generation of consecutive transposes overlaps
            eng = nc.sync if m % 2 == 0 else nc.scalar
            eng.dma_start_transpose(
                o1T_all[:, m * D:(m + 1) * D].rearrange("p (t y) -> p t y", t=4),
                nrm1[m][:])

        h1map = {}

        # strided rhs view: oTr[p, m, t, y] = o1T_all[p, m*512 + t*128 + y]
        oTr = o1T_all[:].rearrange("p (m t y) -> p m t y", m=8, t=4)

        def ffn_h1(h, ft, split=False):
            p1 = ps.tile([128, D], F32, tag="mmA", bufs=2)
            if split:
                # two N=256 halves: the first needs only m-tiles (h*4, h*4+1),
                # so it can start before the later o1T transposes land.
                # half 1 uses start=False on untouched PSUM (has_written unset
                # -> overwrite), so half 0's values are never clobbered.
                for half in range(2):
                    for dt in range(4):
                        nc.tensor.matmul(
                            p1[:, half * 256:(half + 1) * 256],
                            w1_all[:, dt * DFF + ft * 128:
                                   dt * DFF + (ft + 1) * 128],
                            oTr[:, h * 4 + 2 * half:h * 4 + 2 * half + 2,
                                dt, :],
                            start=(half == 0 and dt == 0), stop=(dt == 3),
                            skip_group_check=True)
            else:
                for dt in range(4):
                    nc.tensor.matmul(
                        p1[:],
                        w1_all[:, dt * DFF + ft * 128:dt * DFF + (ft + 1) * 128],
                        oTr[:, h * 4:(h + 1) * 4, dt, :],
                        start=(dt == 0), stop=(dt == 3))
            h1t = hp.tile([128, D], BF16, tag=f"h1_{ft}", bufs=2,
                          name=f"h1_{h}_{ft}")
            nc.scalar.activation(h1t[:], p1[:], AF.Relu,
                                 bias=cf[:, O_B1P + ft:O_B1P + ft + 1])
            h1map[(h, ft)] = h1t

        def ffn_rm(m):
            h, rm = divmod(m, 4)
            tail = m >= 6
            p2 = ps.tile([128, D], F32, tag="mmB", bufs=2)
            for ft in range(16):
                nc.tensor.matmul(
                    p2[:], h1map[(h, ft)][:, rm * 128:(rm + 1) * 128],
                    w2_all[:, ft * D:(ft + 1) * D],
                    start=(ft == 0), stop=(ft == 15 and not tail))
            if tail:
                # z2 += o1 on the (tail-idle) PE; LN2 reads PSUM directly,
                # shortening the post-matmul serial chain of the last tiles
                nc.tensor.matmul(p2[:], EYBcb, o1_sb[m][:],
                                 start=False, stop=True)
                z2 = p2
            else:
                z2t = wk.tile([128, D], BF16, tag="z2")
                nc.vector.tensor_add(z2t[:], p2[:], o1_sb[m][:])
                z2 = z2t
            st6 = wk.tile([128, 6], F32, tag="ls")
            nc.vector.bn_stats(st6[:], z2[:])
            mv = wk.tile([128, 2], F32, tag="lm")
            nc.vector.bn_aggr(mv[:], st6[:])
            sd = wk.tile([128, 1], F32, tag="lsd")
            nc.scalar.activation(sd[:], mv[:, 1:2], AF.Sqrt, bias=epsT[:, :])
            rsd = wk.tile([128, 1], F32, tag="lr")
            nc.vector.reciprocal(rsd[:], sd[:])
            nrm2 = wk.tile([128, D], BF16, tag="n2")
            nc.vector.tensor_scalar(nrm2[:], z2[:], mv[:, 0:1], rsd[:],
                                    op0=ALU.subtract, op1=ALU.mult)
            tg = wk.tile([128, D], BF16, tag="tg")
            yt = wk.tile([128, D], F32, tag="yt")
            if tail:
                nc.vector.tensor_mul(tg[:], nrm2[:], G2cb[:])
                nc.vector.tensor_add(yt[:], tg[:], BE2cb[:])
            else:
                nc.gpsimd.tensor_mul(tg[:], nrm2[:], G2cb[:])
                nc.gpsimd.tensor_add(yt[:], tg[:], BE2cb[:])
            nc.sync.dma_start(out[m * 128:(m + 1) * 128, :], yt[:])

        # ---------------- schedule ----------------
        q_mm_pair(0, 1)
        q_fin(0)
        q_fin(1)
        q_mm(2)
        q_fin(2)
        q_mm(3)
        q_fin(3)
        a0 = attn_softmax(0)
        aT0 = attn_prods(0, a0)
        q_mm(4)
        q_fin(4, copy=False)
        las0 = attn_la(0, aT0)
        q_mm(5)
        q_fin(5, copy=False)
        attn_resid(0, las0, 0)
        attn_resid(0, las0, 1)
        q_mm(6)
        q_mm(7)
        warm_fill(4)
        q_fin(6, copy=False)
        q_fin(7, copy=False)
        attn_resid(0, las0, 2)
        attn_resid(0, las0, 3)
        warm_fill(4)
        q_copy(4)
        q_copy(5)
        a1 = attn_softmax(1)
        aT1 = attn_prods(1, a1)
        q_copy(6)
        q_copy(7)
        warm_fill(4)
        las1 = attn_la(1, aT1)
        attn_resid(1, las1, 0)
        attn_resid(1, las1, 1)
        for ft in range(8):
            ffn_h1(0, ft, split=True)
        attn_resid(1, las1, 2)
        for ft in range(8, 12):
            ffn_h1(0, ft, split=True)
        attn_resid(1, las1, 3)
        for ft in range(12, 16):
            ffn_h1(0, ft, split=True)
        ffn_rm(0)
        for ft in range(0, 4):
            ffn_h1(1, ft)
        ffn_rm(1)
        for ft in range(4, 8):
            ffn_h1(1, ft)
        ffn_rm(2)
        for ft in range(8, 12):
            ffn_h1(1, ft)
        ffn_rm(3)
        for ft in range(12, 16):
            ffn_h1(1, ft)
        for m in range(4, 8):
            ffn_rm(m)

    nc.compile()
    return nc


def _round_f32r(a):
    b = np.ascontiguousarray(a, dtype=np.float32).view(np.uint32)
    out = (b + 0x7FF + ((b >> 12) & 1)) & np.uint32(0xFFFFF000)
    return out.view(np.float32)


def _pe_table():
    pos = np.arange(S, dtype=np.float32)[:, None]
    div = np.exp(np.arange(0, D, 2, dtype=np.float32) * (-math.log(10000.0) / D))
    ang = pos * div
    pe = np.zeros((S, D), np.float32)
    pe[:, 0::2] = np.sin(ang)
    pe[:, 1::2] = np.cos(ang)
    return pe


def make_in_maps(x, Wq, Wfc, W1, b1, W2, b2, g1, be1, g2, be2):
    f32 = lambda a: np.ascontiguousarray(a, dtype=np.float32)
    bfc = lambda a: np.ascontiguousarray(np.asarray(f32(a), dtype="bfloat16"))
    xf = f32(x).reshape(S * H * W, D)
    pe = _pe_table()
    M2 = f32(Wfc).reshape(D, NH, DEP).sum(axis=1).T          # (64, 512)
    Mstk = np.concatenate([M2, M2], axis=0)                  # (128, 512)

    CF = np.zeros((128, NCF), np.float32)
    CF[:, O_EYE:O_EYE + 128] = np.eye(128, dtype=np.float32)
    b1p = f32(b1) + f32(W1) @ f32(be1)                       # be1 folded
    CF[:, O_B1P:O_B1P + 16] = b1p.reshape(16, 128).T

    CB = np.zeros((128, NCB), np.float32)
    CB[:, O_EYB:O_EYB + 128] = np.eye(128, dtype=np.float32)
    CB[:, O_MST:O_MST + D] = Mstk

    CS = np.zeros((8, NCS), np.float32)
    for jt in range(4):
        for p in range(128):
            CS[2 * jt + p // 64, O_E8 + jt * 128 + p] = 1.0
    CS[0, O_G1R:O_G1R + D] = f32(g1)
    CS[0, O_G2R:O_G2R + D] = f32(g2)
    CS[0, O_BE2R:O_BE2R + D] = f32(be2)
    CS[0, O_B1TR:O_B1TR + D] = f32(b2) + f32(be1)

    # selT [8, 128]: selT[j, rr] = 1 iff rr//16 == j; peT per m-tile
    shared = dict(
        WqT=np.ascontiguousarray(f32(Wq.T), dtype=np.float16),
        W1T=bfc(f32(W1) * f32(g1)[None, :]).T.copy(),        # g1 folded
        W2T=bfc(f32(W2).T),
        CF=CF, CB=bfc(CB), CS8=bfc(CS),
    )
    selT = np.zeros((8, 128), np.float32)
    for rr in range(128):
        selT[rr // 16, rr] = 1.0

    maps = []
    for k in range(NCORES):
        sl = xf[k * R:(k + 1) * R]
        m = dict(shared)
        slT = np.asarray(sl.T, dtype=np.float16)
        # xq layout: row-block i = m-pair (2i, 2i+1); columns (dt, mi, c)
        arr = slT.reshape(4, 128, 4, 2, 128)        # (t, p, i, mi, c)
        arr = arr.transpose(2, 1, 0, 3, 4)          # (i, p, t, mi, c)
        m["xT"] = np.ascontiguousarray(arr.reshape(512, 1024))
        # xRb[p, t*512+d] = x[t*128+p, d]
        m["xRb"] = bfc(sl.reshape(8, 128, D).transpose(1, 0, 2)
                       .reshape(128, 8 * D))
        cr8 = np.zeros((8, NCR8), np.float32)
        cr8[:, 0:128] = selT
        pe_loc = pe[k * 64:(k + 1) * 64]            # (64, 512)
        cr8[:, 128:] = (pe_loc.reshape(8, 8, D)
                        .transpose(1, 0, 2).reshape(8, 8 * D))
        m["CR8"] = np.ascontiguousarray(cr8, dtype=np.float16)
        maps.append(m)
    return maps


def kernel(x, Wq, Wfc, W1, b1, W2, b2, g1, be1, g2, be2, _results_hook=None,
           _trace=False, _tmpdir=None):
    if "nc" not in _cached:
        _cached["nc"] = build_nc()
    nc = _cached["nc"]
    in_maps = make_in_maps(x, Wq, Wfc, W1, b1, W2, b2, g1, be1, g2, be2)
    res = run_bass_kernel_spmd(nc, in_maps, list(range(NCORES)),
                               trace=_trace, tmpdir=_tmpdir)
    if _results_hook is not None:
        _results_hook(res)
    y = np.concatenate([res.results[k]["out"] for k in range(NCORES)], axis=0)
    return y.reshape(S, H, W, D)
